# revision 6
# baseline (speedup 1.0000x reference)
"""Trainium2 Bass kernel for nn_CausalAttention (B=4, T=2048, d_model=1024, d_ff=2048).

Sharding: 8 cores = 4 batches x 2 query-halves. Each core owns 8 query blocks
of 128 rows, paired so causal work is balanced and the per-core program is
IDENTICAL (SPMD): the k-th owned block always computes E[k] key chunks; exact
causal masking arrives as per-core input data.

Math (per core, its batch):
  M  = Wq @ Wk.T            (fp32; input-independent)
  uT = (xq M).T             (fp32; owned query rows only)
  S  = uT.T @ x.T  == q @ k.T  (fp32 scores; contraction over d_model=1024
                                instead of d_ff=2048 - half the fp32 matmul work)
  P  = softmax(S + mask)    (max-subtracted, exp on ScalarE, bf16 probs)
  N2 = Wv @ Wf              (bf16; input-independent)
  vf = x @ N2               (bf16, all T rows; spilled to DRAM, streamed back)
  out= P @ vf / rowsum + bf (psum accumulate, scaled+biased in one DVE pass)

fp32 is required through scores: softmax here is unscaled (score std ~45) and
near-one-hot; bf16/tf32 score errors flip argmaxes and corrupt whole rows.

Constraints honored: SBUF pools are a stack allocator (LIFO open/close, pool
footprint = sum of tags, reserved at open); each PSUM accumulation group must
own its 2KB bank region.
"""

import sys
from contextlib import ExitStack

for _p in ("/opt/trn_rl_repo", "/root/.axon_site/_ro/trn_rl_repo"):
    if _p not in sys.path:
        sys.path.append(_p)

import numpy as np

import concourse.bass as bass
import concourse.mybir as mybir
import concourse.tile as tile
from concourse import bacc
from concourse.bass_utils import run_bass_kernel_spmd
from concourse.masks import make_identity

F32 = mybir.dt.float32
BF16 = mybir.dt.bfloat16

B, T, C, F = 4, 2048, 1024, 2048
NB = T // 128  # 16 query/key blocks per batch
CC = C // 128  # 8 chunks of d_model
FC = F // 128  # 16 chunks of d_ff
NCORES = 8

# k-th owned block of each half; chosen so L(OWN_H[h][k]) <= E[k] for both h
# and sum(E)=72 (ideal causal: 68). E[k] = key chunks computed for block k.
OWN_H = {
    0: [15, 12, 11, 8, 7, 4, 3, 0],
    1: [14, 13, 10, 9, 6, 5, 2, 1],
}
E = [16, 14, 12, 10, 8, 6, 4, 2]
NEG = -1.0e30

_CACHE = {}


def _build_program():
    """Trace + finalize the (single, SPMD) Bass program."""
    nc = bacc.Bacc(None)

    x_ext = nc.declare_dram_parameter("x", [T, C], F32, isOutput=False)
    xq_ext = nc.declare_dram_parameter("xq", [1024, C], F32, isOutput=False)
    m2_ext = nc.declare_dram_parameter("mask2", [8, 128, 256], F32, isOutput=False)
    wq_ext = nc.declare_dram_parameter("Wq", [C, F], F32, isOutput=False)
    wk_ext = nc.declare_dram_parameter("Wk", [C, F], F32, isOutput=False)
    wv_ext = nc.declare_dram_parameter("Wv", [C, F], F32, isOutput=False)
    wf_ext = nc.declare_dram_parameter("Wf", [F, F], F32, isOutput=False)
    bf_ext = nc.declare_dram_parameter("bf", [F], F32, isOutput=False)
    out_ext = nc.declare_dram_parameter("out", [8, 128, F], F32, isOutput=True)

    with tile.TileContext(nc) as tc, ExitStack() as root:
        persist = root.enter_context(tc.tile_pool(name="persist", bufs=1))
        ps_t = root.enter_context(tc.tile_pool(name="ps_t", bufs=2, space="PSUM"))
        dram = root.enter_context(tc.tile_pool(name="dram", bufs=1, space="DRAM"))

        ident32 = persist.tile([128, 128], F32, tag="ident32")
        make_identity(nc, ident32[:, :])
        identbf = persist.tile([128, 128], BF16, tag="identbf")
        make_identity(nc, identbf[:, :])
        # long-lived operands: xT 64KB + uT 32KB per partition
        xT = persist.tile([128, CC, T], F32, tag="xT")
        uT = persist.tile([128, CC, 1024], F32, tag="uT")

        def pe_transpose(dst_ap, src_ap, fp32):
            """dst[:,:] = src.T via PE; src/dst are [128,128] APs."""
            pt = ps_t.tile([128, 128], F32 if fp32 else BF16, tag="pt")
            nc.tensor.transpose(
                pt[:, :], src_ap, ident32[:, :] if fp32 else identbf[:, :]
            )
            nc.vector.tensor_copy(out=dst_ap, in_=pt[:, :])

        # ======== phase 1: M = Wq @ Wk.T  [c1, c2] fp32 ====================
        # two passes over halves of the f (d_ff) contraction; second pass adds
        with ExitStack() as phm:
            mxp = phm.enter_context(tc.tile_pool(name="mxp", bufs=1))
            mM = mxp.tile([128, CC, C], F32, tag="M")  # 32KB/part; lives to uT

            with ExitStack() as ph1:
                wqp = ph1.enter_context(tc.tile_pool(name="wqp", bufs=1))
                st1 = ph1.enter_context(tc.tile_pool(name="st1", bufs=2))
                ps1 = ph1.enter_context(tc.tile_pool(name="ps1", bufs=1, space="PSUM"))
                FH = FC // 2  # 8 f-chunks per pass
                for fpass in range(2):
                    # wqT for this half: [f-chunk, c1] fp32, 32KB/part
                    wqTh = wqp.tile([128, FH, C], F32, tag="wqTh")
                    for a in range(CC):
                        wrow = st1.tile([128, 1024], F32, tag="wrow")
                        nc.sync.dma_start(
                            out=wrow,
                            in_=wq_ext[
                                a * 128 : (a + 1) * 128,
                                fpass * 1024 : (fpass + 1) * 1024,
                            ],
                        )
                        for fh in range(FH):
                            pe_transpose(
                                wqTh[:, fh, a * 128 : (a + 1) * 128],
                                wrow[:, fh * 128 : (fh + 1) * 128],
                                True,
                            )
                    for j in range(2):  # c2 strips of 512
                        # wkT cache for (pass, strip): [f-chunk, 512] 16KB/part
                        wkTc = wqp.tile([128, FH, 512], F32, tag="wkTc")
                        for fh in range(FH):
                            for cc in range(4):
                                wkt = st1.tile([128, 128], F32, tag="wkt")
                                r0 = (j * 4 + cc) * 128
                                f0 = (fpass * FH + fh) * 128
                                nc.sync.dma_start(
                                    out=wkt, in_=wk_ext[r0 : r0 + 128, f0 : f0 + 128]
                                )
                                pe_transpose(
                                    wkTc[:, fh, cc * 128 : (cc + 1) * 128], wkt[:, :], True
                                )
                        for ah in range(2):  # c1-chunk halves (4 chunks each)
                            mps = ps1.tile([128, 4, 512], F32, tag="mps")  # 4 banks
                            for fh in range(FH):
                                for a4 in range(4):
                                    nc.tensor.matmul(
                                        mps[:, a4, :],
                                        wqTh[:, fh, (ah * 4 + a4) * 128 : (ah * 4 + a4 + 1) * 128],
                                        wkTc[:, fh, :],
                                        start=(fh == 0),
                                        stop=(fh == FH - 1),
                                    )
                            for a4 in range(4):
                                dst = mM[:, ah * 4 + a4, j * 512 : (j + 1) * 512]
                                if fpass == 0:
                                    nc.vector.tensor_copy(out=dst, in_=mps[:, a4, :])
                                else:
                                    nc.vector.tensor_add(dst, dst, mps[:, a4, :])

            # ======== phase 2: xT (full T) and xqT (owned rows), fp32 ======
            with ExitStack() as ph2:
                xqp = ph2.enter_context(tc.tile_pool(name="xqp", bufs=1))
                st2 = ph2.enter_context(tc.tile_pool(name="st2", bufs=2))
                xqT = xqp.tile([128, CC, 1024], F32, tag="xqT")  # 32KB/part
                for tb in range(NB):
                    xrow = st2.tile([128, C], F32, tag="xrow")
                    nc.sync.dma_start(out=xrow, in_=x_ext[tb * 128 : (tb + 1) * 128, :])
                    for cc in range(CC):
                        pe_transpose(
                            xT[:, cc, tb * 128 : (tb + 1) * 128],
                            xrow[:, cc * 128 : (cc + 1) * 128],
                            True,
                        )
                for tb in range(8):
                    xrow = st2.tile([128, C], F32, tag="xrow")
                    nc.sync.dma_start(out=xrow, in_=xq_ext[tb * 128 : (tb + 1) * 128, :])
                    for cc in range(CC):
                        pe_transpose(
                            xqT[:, cc, tb * 128 : (tb + 1) * 128],
                            xrow[:, cc * 128 : (cc + 1) * 128],
                            True,
                        )

                # ==== phase 3: uT = (xq M).T  [c2-chunk, owned-t] fp32 =====
                with ExitStack() as ph3:
                    ps3 = ph3.enter_context(
                        tc.tile_pool(name="ps3", bufs=2, space="PSUM")
                    )
                    for c2 in range(CC):
                        for tt in range(2):  # owned-t tiles of 512
                            ups = ps3.tile([128, 512], F32, tag="ups")
                            for c1 in range(CC):
                                nc.tensor.matmul(
                                    ups[:, :],
                                    mM[:, c1, c2 * 128 : (c2 + 1) * 128],
                                    xqT[:, c1, tt * 512 : (tt + 1) * 512],
                                    start=(c1 == 0),
                                    stop=(c1 == CC - 1),
                                )
                            nc.vector.tensor_copy(
                                out=uT[:, c2, tt * 512 : (tt + 1) * 512], in_=ups[:, :]
                            )
        # M / xqT pools closed here

        # ======== phase 4: N2 = Wv @ Wf  [c, g] bf16 =======================
        with ExitStack() as phn:
            n2p = phn.enter_context(tc.tile_pool(name="n2p", bufs=1))
            n2 = n2p.tile([128, CC, F], BF16, tag="n2")  # 32KB/part

            with ExitStack() as ph4:
                wvp = ph4.enter_context(tc.tile_pool(name="wvp", bufs=1))
                st4 = ph4.enter_context(tc.tile_pool(name="st4", bufs=2))
                ps4 = ph4.enter_context(tc.tile_pool(name="ps4", bufs=1, space="PSUM"))

                wvT = wvp.tile([128, FC, C], BF16, tag="wvT")  # 32KB/part
                for a in range(CC):
                    for half in range(2):
                        wrow = st4.tile([128, 1024], F32, tag="wrow4")
                        nc.sync.dma_start(
                            out=wrow,
                            in_=wv_ext[
                                a * 128 : (a + 1) * 128, half * 1024 : (half + 1) * 1024
                            ],
                        )
                        wrowb = st4.tile([128, 1024], BF16, tag="wrowb4")
                        nc.vector.tensor_copy(out=wrowb, in_=wrow)
                        for fh in range(FC // 2):
                            f = half * (FC // 2) + fh
                            pe_transpose(
                                wvT[:, f, a * 128 : (a + 1) * 128],
                                wrowb[:, fh * 128 : (fh + 1) * 128],
                                False,
                            )
                # 4 strips of 512 g-columns; Wf slices cached bf16 per strip
                for g in range(4):
                    wfb = wvp.tile([128, FC, 512], BF16, tag="wfb")  # 16KB/part
                    for f in range(FC):
                        wfrow = st4.tile([128, 512], F32, tag="wfrow")
                        nc.sync.dma_start(
                            out=wfrow,
                            in_=wf_ext[
                                f * 128 : (f + 1) * 128, g * 512 : (g + 1) * 512
                            ],
                        )
                        nc.vector.tensor_copy(out=wfb[:, f, :], in_=wfrow)
                    for ah in range(2):
                        nps = ps4.tile([128, 4, 512], F32, tag="nps")  # 4 banks
                        for f in range(FC):
                            for a4 in range(4):
                                nc.tensor.matmul(
                                    nps[:, a4, :],
                                    wvT[:, f, (ah * 4 + a4) * 128 : (ah * 4 + a4 + 1) * 128],
                                    wfb[:, f, :],
                                    start=(f == 0),
                                    stop=(f == FC - 1),
                                )
                        for a4 in range(4):
                            nc.vector.tensor_copy(
                                out=n2[:, ah * 4 + a4, g * 512 : (g + 1) * 512],
                                in_=nps[:, a4, :],
                            )

            # ======== phase 5: vf = x @ N2 -> DRAM (bf16, all T rows) ======
            vf_dram = dram.tile([NB, 128, F], BF16, tag="vf_dram")
            with ExitStack() as ph5:
                st5 = ph5.enter_context(tc.tile_pool(name="st5", bufs=2))
                ps5 = ph5.enter_context(tc.tile_pool(name="ps5", bufs=2, space="PSUM"))
                for tb in range(NB):
                    xtb = st5.tile([128, CC, 128], BF16, tag="xtb")
                    for cc in range(CC):
                        nc.vector.tensor_copy(
                            out=xtb[:, cc, :], in_=xT[:, cc, tb * 128 : (tb + 1) * 128]
                        )
                    vrow = st5.tile([128, F], BF16, tag="vrow")
                    for gh in range(2):  # halves of 1024 g-columns (2 banks each)
                        vps = ps5.tile([128, 1024], F32, tag="vps")
                        for g in range(2):
                            gg = gh * 2 + g
                            for cc in range(CC):
                                nc.tensor.matmul(
                                    vps[:, g * 512 : (g + 1) * 512],
                                    xtb[:, cc, :],
                                    n2[:, cc, gg * 512 : (gg + 1) * 512],
                                    start=(cc == 0),
                                    stop=(cc == CC - 1),
                                )
                        nc.vector.tensor_copy(
                            out=vrow[:, gh * 1024 : (gh + 1) * 1024], in_=vps[:, :]
                        )
                    nc.sync.dma_start(out=vf_dram[tb], in_=vrow)
        # n2 pool closed here

        # ======== phase 6: attention over owned blocks =====================
        with ExitStack() as ph6:
            at = ph6.enter_context(tc.tile_pool(name="at", bufs=1))
            st6 = ph6.enter_context(tc.tile_pool(name="st6", bufs=2))
            vrd = ph6.enter_context(tc.tile_pool(name="vrd", bufs=4))
            small = ph6.enter_context(tc.tile_pool(name="small", bufs=4))
            ps6 = ph6.enter_context(tc.tile_pool(name="ps6", bufs=1, space="PSUM"))

            bfb = at.tile([128, F], F32, tag="bfb")
            bf_ap = bf_ext[:]
            nc.sync.dma_start(
                out=bfb,
                in_=bass.AP(
                    tensor=bf_ap.tensor,
                    offset=bf_ap.offset,
                    ap=[[0, 128]] + list(bf_ap.ap),
                ),
            )
            m2 = at.tile([128, 8, 256], F32, tag="m2")
            for k in range(8):
                nc.sync.dma_start(out=m2[:, k, :], in_=m2_ext[k])

            for k in range(8):
                ek = E[k]
                scols = ek * 128
                # scores S = uT[k].T @ xT  (fp32) into shared 4-bank psum slot
                sps = ps6.tile([128, T], F32, tag="big")
                for st in range(0, scols, 512):
                    w = min(512, scols - st)
                    for c2 in range(CC):
                        nc.tensor.matmul(
                            sps[:, st : st + w],
                            uT[:, c2, k * 128 : (k + 1) * 128],
                            xT[:, c2, st : st + w],
                            start=(c2 == 0),
                            stop=(c2 == CC - 1),
                        )
                # move scores to SBUF (frees the psum slot for the out matmuls);
                # fuse the causal mask add (last two chunks) into the copy
                s_sb = st6.tile([128, T], F32, tag="s_sb")
                if scols > 256:
                    nc.vector.tensor_copy(
                        out=s_sb[:, : scols - 256], in_=sps[:, : scols - 256]
                    )
                nc.vector.tensor_add(
                    s_sb[:, scols - 256 : scols],
                    sps[:, scols - 256 : scols],
                    m2[:, k, :],
                )
                negmax = small.tile([128, 1], F32, tag="negmax")
                nc.vector.tensor_reduce(
                    out=negmax,
                    in_=s_sb[:, :scols],
                    axis=mybir.AxisListType.X,
                    op=mybir.AluOpType.max,
                    negate=True,
                )
                psb = st6.tile([128, T], BF16, tag="psb")
                rsum = small.tile([128, 1], F32, tag="rsum")
                nc.scalar.activation(
                    out=psb[:, :scols],
                    in_=s_sb[:, :scols],
                    func=mybir.ActivationFunctionType.Exp,
                    bias=negmax,
                    scale=1.0,
                    accum_out=rsum,
                )
                rinv = small.tile([128, 1], F32, tag="rinv")
                nc.vector.reciprocal(out=rinv, in_=rsum)
                # transpose probabilities: PT[s-chunk] = P[:, sc].T (bf16)
                ptsb = st6.tile([128, NB, 128], BF16, tag="ptsb")
                for sc in range(ek):
                    pe_transpose(
                        ptsb[:, sc, :], psb[:, sc * 128 : (sc + 1) * 128], False
                    )
                # out = P @ vf (accumulate over s-chunks), then /rowsum + bf
                ops = ps6.tile([128, F], F32, tag="big")  # reuses the slot
                for sc in range(ek):
                    vrow = vrd.tile([128, F], BF16, tag="vread")
                    nc.sync.dma_start(out=vrow, in_=vf_dram[sc])
                    for g in range(4):
                        nc.tensor.matmul(
                            ops[:, g * 512 : (g + 1) * 512],
                            ptsb[:, sc, :],
                            vrow[:, g * 512 : (g + 1) * 512],
                            start=(sc == 0),
                            stop=(sc == ek - 1),
                        )
                orow = st6.tile([128, F], F32, tag="orow")
                nc.vector.scalar_tensor_tensor(
                    out=orow,
                    in0=ops,
                    scalar=rinv,
                    in1=bfb,
                    op0=mybir.AluOpType.mult,
                    op1=mybir.AluOpType.add,
                )
                nc.sync.dma_start(out=out_ext[k], in_=orow)

    nc.finalize()
    return nc


def _get_program():
    if "nc" not in _CACHE:
        _CACHE["nc"] = _build_program()
    return _CACHE["nc"]


def _make_in_maps(x, Wq, Wk, Wv, Wf, bf):
    x = np.ascontiguousarray(x, dtype=np.float32)
    in_maps = []
    for core in range(NCORES):
        b, h = core // 2, core % 2
        own = OWN_H[h]
        xb = x[b]
        xq = np.concatenate([xb[blk * 128 : (blk + 1) * 128] for blk in own], axis=0)
        mask2 = np.zeros((8, 128, 256), dtype=np.float32)
        for k, blk in enumerate(own):
            s0 = (E[k] - 2) * 128  # global key index of mask window start
            s = s0 + np.arange(256)[None, :]
            t = blk * 128 + np.arange(128)[:, None]
            mask2[k] = np.where(s <= t, 0.0, NEG).astype(np.float32)
        in_maps.append(
            {
                "x": np.ascontiguousarray(xb),
                "xq": np.ascontiguousarray(xq),
                "mask2": mask2,
                "Wq": np.ascontiguousarray(Wq, dtype=np.float32),
                "Wk": np.ascontiguousarray(Wk, dtype=np.float32),
                "Wv": np.ascontiguousarray(Wv, dtype=np.float32),
                "Wf": np.ascontiguousarray(Wf, dtype=np.float32),
                "bf": np.ascontiguousarray(bf, dtype=np.float32),
            }
        )
    return in_maps


def run_on_hw(inputs, trace=False, trace_cores=None):
    nc = _get_program()
    in_maps = _make_in_maps(**inputs)
    res = run_bass_kernel_spmd(
        nc, in_maps, list(range(NCORES)), trace=trace, trace_cores=trace_cores
    )
    out = np.empty((B, T, F), dtype=np.float32)
    for core in range(NCORES):
        b, h = core // 2, core % 2
        own = OWN_H[h]
        o = res.results[core]["out"]  # [8, 128, F]
        for k, blk in enumerate(own):
            out[b, blk * 128 : (blk + 1) * 128, :] = o[k]
    return out, res


def kernel(x, Wq, Wk, Wv, Wf, bf):
    out, _ = run_on_hw(dict(x=x, Wq=Wq, Wk=Wk, Wv=Wv, Wf=Wf, bf=bf))
    return out


# revision 9
# speedup vs baseline: 1.0832x; 1.0832x over previous
"""Trainium2 Bass kernel for nn_CausalAttention (B=4, T=2048, d_model=1024, d_ff=2048).

Sharding: 8 cores = 4 batches x 2 query-halves. Each core owns 8 query blocks
of 128 rows, paired so causal work is balanced and the per-core program is
IDENTICAL (SPMD): the k-th owned block always computes E[k] key chunks; exact
causal masking arrives as per-core input data.

Math (per core, its batch):
  M  = Wq @ Wk.T            (fp32; input-independent)
  uT = (xq M).T             (fp32; owned query rows only)
  S  = uT.T @ x.T  == q @ k.T  (fp32 scores; contraction over d_model=1024
                                instead of d_ff=2048 - half the fp32 matmul work)
  P  = softmax(S + mask)    (max-subtracted, exp on ScalarE, bf16 probs)
  N2 = Wv @ Wf              (bf16; input-independent)
  vf = x @ N2               (bf16, all T rows; spilled to DRAM, streamed back)
  out= P @ vf / rowsum + bf (psum accumulate, scaled+biased in one DVE pass)

fp32 is required through scores: softmax here is unscaled (score std ~45) and
near-one-hot; bf16/tf32 score errors flip argmaxes and corrupt whole rows.

Constraints honored: SBUF pools are a stack allocator (LIFO open/close, pool
footprint = sum of tags, reserved at open); each PSUM accumulation group must
own its 2KB bank region.
"""

import sys
from contextlib import ExitStack

for _p in ("/opt/trn_rl_repo", "/root/.axon_site/_ro/trn_rl_repo"):
    if _p not in sys.path:
        sys.path.append(_p)

import numpy as np

import concourse.bass as bass
import concourse.mybir as mybir
import concourse.tile as tile
from concourse import bacc
from concourse.bass_utils import run_bass_kernel_spmd
from concourse.masks import make_identity

F32 = mybir.dt.float32
BF16 = mybir.dt.bfloat16

B, T, C, F = 4, 2048, 1024, 2048
NB = T // 128  # 16 query/key blocks per batch
CC = C // 128  # 8 chunks of d_model
FC = F // 128  # 16 chunks of d_ff
NCORES = 8

# k-th owned block of each half; chosen so L(OWN_H[h][k]) <= E[k] for both h
# and sum(E)=72 (ideal causal: 68). E[k] = key chunks computed for block k.
OWN_H = {
    0: [15, 12, 11, 8, 7, 4, 3, 0],
    1: [14, 13, 10, 9, 6, 5, 2, 1],
}
E = [16, 14, 12, 10, 8, 6, 4, 2]
NEG = -1.0e30

_CACHE = {}


def _build_program():
    """Trace + finalize the (single, SPMD) Bass program."""
    nc = bacc.Bacc(None)

    x_ext = nc.declare_dram_parameter("x", [T, C], F32, isOutput=False)
    xq_ext = nc.declare_dram_parameter("xq", [1024, C], F32, isOutput=False)
    m2_ext = nc.declare_dram_parameter("mask2", [8, 128, 256], F32, isOutput=False)
    wq_ext = nc.declare_dram_parameter("Wq", [C, F], F32, isOutput=False)
    wk_ext = nc.declare_dram_parameter("Wk", [C, F], F32, isOutput=False)
    wv_ext = nc.declare_dram_parameter("Wv", [C, F], F32, isOutput=False)
    wf_ext = nc.declare_dram_parameter("Wf", [F, F], F32, isOutput=False)
    bf_ext = nc.declare_dram_parameter("bf", [F], F32, isOutput=False)
    out_ext = nc.declare_dram_parameter("out", [8, 128, F], F32, isOutput=True)

    with tile.TileContext(nc) as tc, ExitStack() as root:
        persist = root.enter_context(tc.tile_pool(name="persist", bufs=1))
        ps_t = root.enter_context(tc.tile_pool(name="ps_t", bufs=2, space="PSUM"))
        dram = root.enter_context(tc.tile_pool(name="dram", bufs=1, space="DRAM"))

        ident32 = persist.tile([128, 128], F32, tag="ident32")
        make_identity(nc, ident32[:, :])
        identbf = persist.tile([128, 128], BF16, tag="identbf")
        make_identity(nc, identbf[:, :])
        # long-lived operands: xT 64KB + uT 32KB per partition
        xT = persist.tile([128, CC, T], F32, tag="xT")
        uT = persist.tile([128, CC, 1024], F32, tag="uT")

        def pe_transpose(dst_ap, src_ap, fp32):
            """dst[:,:] = src.T via PE; src/dst are [128,128] APs."""
            pt = ps_t.tile([128, 128], F32 if fp32 else BF16, tag="pt")
            nc.tensor.transpose(
                pt[:, :], src_ap, ident32[:, :] if fp32 else identbf[:, :]
            )
            nc.vector.tensor_copy(out=dst_ap, in_=pt[:, :])

        # ======== phase 1: M = Wq @ Wk.T  [c1, c2] fp32 ====================
        # two passes over halves of the f (d_ff) contraction; second pass adds
        with ExitStack() as phm:
            mxp = phm.enter_context(tc.tile_pool(name="mxp", bufs=1))
            mM = mxp.tile([128, CC, C], F32, tag="M")  # 32KB/part; lives to uT

            with ExitStack() as ph1:
                wqp = ph1.enter_context(tc.tile_pool(name="wqp", bufs=1))
                st1 = ph1.enter_context(tc.tile_pool(name="st1", bufs=2))
                ps1 = ph1.enter_context(tc.tile_pool(name="ps1", bufs=1, space="PSUM"))
                FH = FC // 2  # 8 f-chunks per pass
                for fpass in range(2):
                    # wqT for this half: [f-chunk, c1] fp32, 32KB/part
                    wqTh = wqp.tile([128, FH, C], F32, tag="wqTh")
                    for a in range(CC):
                        wrow = st1.tile([128, 1024], F32, tag="wrow")
                        nc.sync.dma_start(
                            out=wrow,
                            in_=wq_ext[
                                a * 128 : (a + 1) * 128,
                                fpass * 1024 : (fpass + 1) * 1024,
                            ],
                        )
                        for fh in range(FH):
                            pe_transpose(
                                wqTh[:, fh, a * 128 : (a + 1) * 128],
                                wrow[:, fh * 128 : (fh + 1) * 128],
                                True,
                            )
                    for j in range(2):  # c2 strips of 512
                        # wkT cache for (pass, strip): [f-chunk, 512] 16KB/part
                        wkTc = wqp.tile([128, FH, 512], F32, tag="wkTc")
                        for fh in range(FH):
                            for cc in range(4):
                                wkt = st1.tile([128, 128], F32, tag="wkt")
                                r0 = (j * 4 + cc) * 128
                                f0 = (fpass * FH + fh) * 128
                                nc.sync.dma_start(
                                    out=wkt, in_=wk_ext[r0 : r0 + 128, f0 : f0 + 128]
                                )
                                pe_transpose(
                                    wkTc[:, fh, cc * 128 : (cc + 1) * 128], wkt[:, :], True
                                )
                        for ah in range(2):  # c1-chunk halves (4 chunks each)
                            mps = ps1.tile([128, 4, 512], F32, tag="mps")  # 4 banks
                            for fh in range(FH):
                                for a4 in range(4):
                                    nc.tensor.matmul(
                                        mps[:, a4, :],
                                        wqTh[:, fh, (ah * 4 + a4) * 128 : (ah * 4 + a4 + 1) * 128],
                                        wkTc[:, fh, :],
                                        start=(fh == 0),
                                        stop=(fh == FH - 1),
                                    )
                            for a4 in range(4):
                                dst = mM[:, ah * 4 + a4, j * 512 : (j + 1) * 512]
                                if fpass == 0:
                                    nc.vector.tensor_copy(out=dst, in_=mps[:, a4, :])
                                else:
                                    nc.vector.tensor_add(dst, dst, mps[:, a4, :])

            # ======== phase 2: xT (full T) and xqT (owned rows), fp32 ======
            with ExitStack() as ph2:
                xqp = ph2.enter_context(tc.tile_pool(name="xqp", bufs=1))
                st2 = ph2.enter_context(tc.tile_pool(name="st2", bufs=2))
                xqT = xqp.tile([128, CC, 1024], F32, tag="xqT")  # 32KB/part
                for tb in range(NB):
                    xrow = st2.tile([128, C], F32, tag="xrow")
                    nc.sync.dma_start(out=xrow, in_=x_ext[tb * 128 : (tb + 1) * 128, :])
                    for cc in range(CC):
                        pe_transpose(
                            xT[:, cc, tb * 128 : (tb + 1) * 128],
                            xrow[:, cc * 128 : (cc + 1) * 128],
                            True,
                        )
                for tb in range(8):
                    xrow = st2.tile([128, C], F32, tag="xrow")
                    nc.sync.dma_start(out=xrow, in_=xq_ext[tb * 128 : (tb + 1) * 128, :])
                    for cc in range(CC):
                        pe_transpose(
                            xqT[:, cc, tb * 128 : (tb + 1) * 128],
                            xrow[:, cc * 128 : (cc + 1) * 128],
                            True,
                        )

                # ==== phase 3: uT = (xq M).T  [c2-chunk, owned-t] fp32 =====
                with ExitStack() as ph3:
                    ps3 = ph3.enter_context(
                        tc.tile_pool(name="ps3", bufs=2, space="PSUM")
                    )
                    for c2 in range(CC):
                        for tt in range(2):  # owned-t tiles of 512
                            ups = ps3.tile([128, 512], F32, tag="ups")
                            for c1 in range(CC):
                                nc.tensor.matmul(
                                    ups[:, :],
                                    mM[:, c1, c2 * 128 : (c2 + 1) * 128],
                                    xqT[:, c1, tt * 512 : (tt + 1) * 512],
                                    start=(c1 == 0),
                                    stop=(c1 == CC - 1),
                                )
                            nc.vector.tensor_copy(
                                out=uT[:, c2, tt * 512 : (tt + 1) * 512], in_=ups[:, :]
                            )
        # M / xqT pools closed here

        # ======== phase 4: N2 = Wv @ Wf  [c, g] bf16 =======================
        with ExitStack() as phn:
            n2p = phn.enter_context(tc.tile_pool(name="n2p", bufs=1))
            n2 = n2p.tile([128, CC, F], BF16, tag="n2")  # 32KB/part

            with ExitStack() as ph4:
                wvp = ph4.enter_context(tc.tile_pool(name="wvp", bufs=1))
                st4 = ph4.enter_context(tc.tile_pool(name="st4", bufs=2))
                ps4 = ph4.enter_context(tc.tile_pool(name="ps4", bufs=1, space="PSUM"))

                wvT = wvp.tile([128, FC, C], BF16, tag="wvT")  # 32KB/part
                for a in range(CC):
                    for half in range(2):
                        wrow = st4.tile([128, 1024], F32, tag="wrow4")
                        nc.sync.dma_start(
                            out=wrow,
                            in_=wv_ext[
                                a * 128 : (a + 1) * 128, half * 1024 : (half + 1) * 1024
                            ],
                        )
                        wrowb = st4.tile([128, 1024], BF16, tag="wrowb4")
                        nc.vector.tensor_copy(out=wrowb, in_=wrow)
                        for fh in range(FC // 2):
                            f = half * (FC // 2) + fh
                            pe_transpose(
                                wvT[:, f, a * 128 : (a + 1) * 128],
                                wrowb[:, fh * 128 : (fh + 1) * 128],
                                False,
                            )
                # 4 strips of 512 g-columns; Wf slices cached bf16 per strip
                for g in range(4):
                    wfb = wvp.tile([128, FC, 512], BF16, tag="wfb")  # 16KB/part
                    for f in range(FC):
                        wfrow = st4.tile([128, 512], F32, tag="wfrow")
                        nc.sync.dma_start(
                            out=wfrow,
                            in_=wf_ext[
                                f * 128 : (f + 1) * 128, g * 512 : (g + 1) * 512
                            ],
                        )
                        nc.vector.tensor_copy(out=wfb[:, f, :], in_=wfrow)
                    for ah in range(2):
                        nps = ps4.tile([128, 4, 512], F32, tag="nps")  # 4 banks
                        for f in range(FC):
                            for a4 in range(4):
                                nc.tensor.matmul(
                                    nps[:, a4, :],
                                    wvT[:, f, (ah * 4 + a4) * 128 : (ah * 4 + a4 + 1) * 128],
                                    wfb[:, f, :],
                                    start=(f == 0),
                                    stop=(f == FC - 1),
                                )
                        for a4 in range(4):
                            nc.vector.tensor_copy(
                                out=n2[:, ah * 4 + a4, g * 512 : (g + 1) * 512],
                                in_=nps[:, a4, :],
                            )

            # ======== phase 5: vf = x @ N2 -> DRAM (bf16, all T rows) ======
            vf_dram = dram.tile([NB, 128, F], BF16, tag="vf_dram")
            with ExitStack() as ph5:
                st5 = ph5.enter_context(tc.tile_pool(name="st5", bufs=2))
                ps5 = ph5.enter_context(tc.tile_pool(name="ps5", bufs=2, space="PSUM"))
                for tb in range(NB):
                    xtb = st5.tile([128, CC, 128], BF16, tag="xtb")
                    for cc in range(CC):
                        nc.vector.tensor_copy(
                            out=xtb[:, cc, :], in_=xT[:, cc, tb * 128 : (tb + 1) * 128]
                        )
                    vrow = st5.tile([128, F], BF16, tag="vrow")
                    for gh in range(2):  # halves of 1024 g-columns (2 banks each)
                        vps = ps5.tile([128, 1024], F32, tag="vps")
                        for g in range(2):
                            gg = gh * 2 + g
                            for cc in range(CC):
                                nc.tensor.matmul(
                                    vps[:, g * 512 : (g + 1) * 512],
                                    xtb[:, cc, :],
                                    n2[:, cc, gg * 512 : (gg + 1) * 512],
                                    start=(cc == 0),
                                    stop=(cc == CC - 1),
                                )
                        nc.vector.tensor_copy(
                            out=vrow[:, gh * 1024 : (gh + 1) * 1024], in_=vps[:, :]
                        )
                    nc.sync.dma_start(out=vf_dram[tb], in_=vrow)
        # n2 pool closed here

        # ======== phase 6: attention over owned blocks =====================
        with ExitStack() as ph6:
            at = ph6.enter_context(tc.tile_pool(name="at", bufs=1))
            st6 = ph6.enter_context(tc.tile_pool(name="st6", bufs=2))
            vrd = ph6.enter_context(tc.tile_pool(name="vrd", bufs=4))
            small = ph6.enter_context(tc.tile_pool(name="small", bufs=4))
            ps6 = ph6.enter_context(tc.tile_pool(name="ps6", bufs=1, space="PSUM"))

            bfb = at.tile([128, F], F32, tag="bfb")
            bf_ap = bf_ext[:]
            nc.sync.dma_start(
                out=bfb,
                in_=bass.AP(
                    tensor=bf_ap.tensor,
                    offset=bf_ap.offset,
                    ap=[[0, 128]] + list(bf_ap.ap),
                ),
            )
            m2 = at.tile([128, 8, 256], F32, tag="m2")
            for k in range(8):
                nc.sync.dma_start(out=m2[:, k, :], in_=m2_ext[k])

            for k in range(8):
                ek = E[k]
                scols = ek * 128
                # scores S = uT[k].T @ xT (fp32), computed in 2-bank psum
                # halves so the next block's scores overlap this block's
                # out-matmuls; each half lands in SBUF with the causal mask
                # (last two chunks) fused into the copy
                s_sb = st6.tile([128, T], F32, tag="s_sb")
                for h0 in range(0, scols, 1024):
                    hw = min(1024, scols - h0)
                    sps = ps6.tile([128, 1024], F32, tag="sps")  # 2 banks
                    for st in range(0, hw, 512):
                        w = min(512, hw - st)
                        for c2 in range(CC):
                            nc.tensor.matmul(
                                sps[:, st : st + w],
                                uT[:, c2, k * 128 : (k + 1) * 128],
                                xT[:, c2, h0 + st : h0 + st + w],
                                start=(c2 == 0),
                                stop=(c2 == CC - 1),
                            )
                    m0 = scols - 256  # mask window start
                    plain = min(hw, max(0, m0 - h0))
                    if plain > 0:
                        nc.vector.tensor_copy(
                            out=s_sb[:, h0 : h0 + plain], in_=sps[:, :plain]
                        )
                    if plain < hw:
                        nc.vector.tensor_add(
                            s_sb[:, h0 + plain : h0 + hw],
                            sps[:, plain:hw],
                            m2[:, k, h0 + plain - m0 : h0 + hw - m0],
                        )
                negmax = small.tile([128, 1], F32, tag="negmax")
                nc.vector.tensor_reduce(
                    out=negmax,
                    in_=s_sb[:, :scols],
                    axis=mybir.AxisListType.X,
                    op=mybir.AluOpType.max,
                    negate=True,
                )
                psb = st6.tile([128, T], BF16, tag="psb")
                rsum = small.tile([128, 1], F32, tag="rsum")
                nc.scalar.activation(
                    out=psb[:, :scols],
                    in_=s_sb[:, :scols],
                    func=mybir.ActivationFunctionType.Exp,
                    bias=negmax,
                    scale=1.0,
                    accum_out=rsum,
                )
                rinv = small.tile([128, 1], F32, tag="rinv")
                nc.vector.reciprocal(out=rinv, in_=rsum)
                # transpose probabilities: PT[s-chunk] = P[:, sc].T (bf16)
                ptsb = st6.tile([128, NB, 128], BF16, tag="ptsb")
                for sc in range(ek):
                    pe_transpose(
                        ptsb[:, sc, :], psb[:, sc * 128 : (sc + 1) * 128], False
                    )
                # out = P @ vf (accumulate over s-chunks), then /rowsum + bf
                ops = ps6.tile([128, F], F32, tag="ops")  # 4 banks
                for sc in range(ek):
                    vrow = vrd.tile([128, F], BF16, tag="vread")
                    nc.sync.dma_start(out=vrow, in_=vf_dram[sc])
                    for g in range(4):
                        nc.tensor.matmul(
                            ops[:, g * 512 : (g + 1) * 512],
                            ptsb[:, sc, :],
                            vrow[:, g * 512 : (g + 1) * 512],
                            start=(sc == 0),
                            stop=(sc == ek - 1),
                        )
                orow = st6.tile([128, F], F32, tag="orow")
                nc.vector.scalar_tensor_tensor(
                    out=orow,
                    in0=ops,
                    scalar=rinv,
                    in1=bfb,
                    op0=mybir.AluOpType.mult,
                    op1=mybir.AluOpType.add,
                )
                nc.sync.dma_start(out=out_ext[k], in_=orow)

    nc.finalize()
    return nc


def _get_program():
    if "nc" not in _CACHE:
        _CACHE["nc"] = _build_program()
    return _CACHE["nc"]


def _make_in_maps(x, Wq, Wk, Wv, Wf, bf):
    x = np.ascontiguousarray(x, dtype=np.float32)
    in_maps = []
    for core in range(NCORES):
        b, h = core // 2, core % 2
        own = OWN_H[h]
        xb = x[b]
        xq = np.concatenate([xb[blk * 128 : (blk + 1) * 128] for blk in own], axis=0)
        mask2 = np.zeros((8, 128, 256), dtype=np.float32)
        for k, blk in enumerate(own):
            s0 = (E[k] - 2) * 128  # global key index of mask window start
            s = s0 + np.arange(256)[None, :]
            t = blk * 128 + np.arange(128)[:, None]
            mask2[k] = np.where(s <= t, 0.0, NEG).astype(np.float32)
        in_maps.append(
            {
                "x": np.ascontiguousarray(xb),
                "xq": np.ascontiguousarray(xq),
                "mask2": mask2,
                "Wq": np.ascontiguousarray(Wq, dtype=np.float32),
                "Wk": np.ascontiguousarray(Wk, dtype=np.float32),
                "Wv": np.ascontiguousarray(Wv, dtype=np.float32),
                "Wf": np.ascontiguousarray(Wf, dtype=np.float32),
                "bf": np.ascontiguousarray(bf, dtype=np.float32),
            }
        )
    return in_maps


def run_on_hw(inputs, trace=False, trace_cores=None):
    nc = _get_program()
    in_maps = _make_in_maps(**inputs)
    res = run_bass_kernel_spmd(
        nc, in_maps, list(range(NCORES)), trace=trace, trace_cores=trace_cores
    )
    out = np.empty((B, T, F), dtype=np.float32)
    for core in range(NCORES):
        b, h = core // 2, core % 2
        own = OWN_H[h]
        o = res.results[core]["out"]  # [8, 128, F]
        for k, blk in enumerate(own):
            out[b, blk * 128 : (blk + 1) * 128, :] = o[k]
    return out, res


def kernel(x, Wq, Wk, Wv, Wf, bf):
    out, _ = run_on_hw(dict(x=x, Wq=Wq, Wk=Wk, Wv=Wv, Wf=Wf, bf=bf))
    return out


# revision 10
# speedup vs baseline: 1.4034x; 1.2956x over previous
"""Trainium2 Bass kernel for nn_CausalAttention (B=4, T=2048, d_model=1024, d_ff=2048).

Sharding: 8 cores = 4 batches x 2 query-halves. Each core owns 8 query blocks
of 128 rows, paired so causal work is balanced and the per-core program is
IDENTICAL (SPMD): the k-th owned block always computes E[k] key chunks; exact
causal masking arrives as per-core input data. Host-side input marshalling
ships operands pre-transposed (and bf16-cast where allowed) so the device
spends no PE/DVE time on layout.

Math (per core, its batch):
  M  = Wq @ Wk.T            (fp32; input-independent)
  uT = (xq M).T             (fp32; owned query rows only)
  S  = uT.T @ x.T  == q @ k.T  (fp32 scores; contraction over d_model=1024
                                instead of d_ff=2048 - half the fp32 matmul work)
  P  = softmax(S + mask)    (max-subtracted, exp on ScalarE, bf16 probs)
  N2 = Wv @ Wf              (bf16; input-independent)
  vf = x @ N2               (bf16, all T rows; spilled to DRAM, streamed back)
  out= P @ vf / rowsum + bf (psum accumulate, scaled+biased in one DVE pass)

fp32 is required through scores: softmax here is unscaled (score std ~45) and
near-one-hot; bf16/tf32 score errors flip argmaxes and corrupt whole rows.

Constraints honored: SBUF pools are a stack allocator (LIFO open/close, pool
footprint = sum of tags, reserved at open); each PSUM accumulation group must
own its 2KB bank region.
"""

import sys
from contextlib import ExitStack

for _p in ("/opt/trn_rl_repo", "/root/.axon_site/_ro/trn_rl_repo"):
    if _p not in sys.path:
        sys.path.append(_p)

import ml_dtypes
import numpy as np

import concourse.bass as bass
import concourse.mybir as mybir
import concourse.tile as tile
from concourse import bacc
from concourse.bass_utils import run_bass_kernel_spmd
from concourse.masks import make_identity

F32 = mybir.dt.float32
BF16 = mybir.dt.bfloat16

B, T, C, F = 4, 2048, 1024, 2048
NB = T // 128  # 16 query/key blocks per batch
CC = C // 128  # 8 chunks of d_model
FC = F // 128  # 16 chunks of d_ff
NCORES = 8

# k-th owned block of each half; chosen so L(OWN_H[h][k]) <= E[k] for both h
# and sum(E)=72 (ideal causal: 68). E[k] = key chunks computed for block k.
OWN_H = {
    0: [15, 12, 11, 8, 7, 4, 3, 0],
    1: [14, 13, 10, 9, 6, 5, 2, 1],
}
E = [16, 14, 12, 10, 8, 6, 4, 2]
NEG = -1.0e30

_CACHE = {}


def _build_program():
    """Trace + finalize the (single, SPMD) Bass program."""
    nc = bacc.Bacc(None)

    # all operands arrive pre-transposed / pre-cast from the host
    xT_ext = nc.declare_dram_parameter("xTin", [C, T], F32, isOutput=False)
    xqT_ext = nc.declare_dram_parameter("xqTin", [C, 1024], F32, isOutput=False)
    m2_ext = nc.declare_dram_parameter("mask2", [8, 128, 256], F32, isOutput=False)
    wqT_ext = nc.declare_dram_parameter("WqT", [F, C], F32, isOutput=False)
    wkT_ext = nc.declare_dram_parameter("WkT", [F, C], F32, isOutput=False)
    wvT_ext = nc.declare_dram_parameter("WvTb", [F, C], BF16, isOutput=False)
    wf_ext = nc.declare_dram_parameter("Wfb", [F, F], BF16, isOutput=False)
    bf_ext = nc.declare_dram_parameter("bf", [F], F32, isOutput=False)
    out_ext = nc.declare_dram_parameter("out", [8, 128, F], F32, isOutput=True)

    with tile.TileContext(nc) as tc, ExitStack() as root:
        persist = root.enter_context(tc.tile_pool(name="persist", bufs=1))
        ps_t = root.enter_context(tc.tile_pool(name="ps_t", bufs=2, space="PSUM"))
        dram = root.enter_context(tc.tile_pool(name="dram", bufs=1, space="DRAM"))

        identbf = persist.tile([128, 128], BF16, tag="identbf")
        make_identity(nc, identbf[:, :])
        # long-lived operands: xT 64KB + uT 32KB per partition
        xT = persist.tile([128, CC, T], F32, tag="xT")
        for cc in range(CC):
            nc.sync.dma_start(out=xT[:, cc, :], in_=xT_ext[cc * 128 : (cc + 1) * 128, :])
        uT = persist.tile([128, CC, 1024], F32, tag="uT")

        # ======== phase 1: M = Wq @ Wk.T  [c1, c2] fp32 ====================
        # two passes over halves of the f (d_ff) contraction; second pass adds
        with ExitStack() as phm:
            mxp = phm.enter_context(tc.tile_pool(name="mxp", bufs=1))
            mM = mxp.tile([128, CC, C], F32, tag="M")  # 32KB/part; lives to uT

            with ExitStack() as ph1:
                wqp = ph1.enter_context(tc.tile_pool(name="wqp", bufs=1))
                ps1 = ph1.enter_context(tc.tile_pool(name="ps1", bufs=1, space="PSUM"))
                FH = FC // 2  # 8 f-chunks per pass
                for fpass in range(2):
                    # wqT for this half: [f-chunk, c1] fp32, 32KB/part
                    wqTh = wqp.tile([128, FH, C], F32, tag="wqTh")
                    for fh in range(FH):
                        f0 = (fpass * FH + fh) * 128
                        nc.sync.dma_start(
                            out=wqTh[:, fh, :], in_=wqT_ext[f0 : f0 + 128, :]
                        )
                    for j in range(2):  # c2 strips of 512
                        # wkT cache for (pass, strip): [f-chunk, 512] 16KB/part
                        wkTc = wqp.tile([128, FH, 512], F32, tag="wkTc")
                        for fh in range(FH):
                            f0 = (fpass * FH + fh) * 128
                            nc.sync.dma_start(
                                out=wkTc[:, fh, :],
                                in_=wkT_ext[f0 : f0 + 128, j * 512 : (j + 1) * 512],
                            )
                        for ah in range(2):  # c1-chunk halves (4 chunks each)
                            mps = ps1.tile([128, 4, 512], F32, tag="mps")  # 4 banks
                            for fh in range(FH):
                                for a4 in range(4):
                                    nc.tensor.matmul(
                                        mps[:, a4, :],
                                        wqTh[:, fh, (ah * 4 + a4) * 128 : (ah * 4 + a4 + 1) * 128],
                                        wkTc[:, fh, :],
                                        start=(fh == 0),
                                        stop=(fh == FH - 1),
                                    )
                            for a4 in range(4):
                                dst = mM[:, ah * 4 + a4, j * 512 : (j + 1) * 512]
                                if fpass == 0:
                                    nc.vector.tensor_copy(out=dst, in_=mps[:, a4, :])
                                else:
                                    nc.vector.tensor_add(dst, dst, mps[:, a4, :])

            # ======== phase 2+3: uT = (xq M).T  [c2-chunk, owned-t] fp32 ===
            with ExitStack() as ph2:
                xqp = ph2.enter_context(tc.tile_pool(name="xqp", bufs=1))
                xqT = xqp.tile([128, CC, 1024], F32, tag="xqT")  # 32KB/part
                for cc in range(CC):
                    nc.sync.dma_start(
                        out=xqT[:, cc, :], in_=xqT_ext[cc * 128 : (cc + 1) * 128, :]
                    )
                with ExitStack() as ph3:
                    ps3 = ph3.enter_context(
                        tc.tile_pool(name="ps3", bufs=2, space="PSUM")
                    )
                    for c2 in range(CC):
                        for tt in range(2):  # owned-t tiles of 512
                            ups = ps3.tile([128, 512], F32, tag="ups")
                            for c1 in range(CC):
                                nc.tensor.matmul(
                                    ups[:, :],
                                    mM[:, c1, c2 * 128 : (c2 + 1) * 128],
                                    xqT[:, c1, tt * 512 : (tt + 1) * 512],
                                    start=(c1 == 0),
                                    stop=(c1 == CC - 1),
                                )
                            nc.vector.tensor_copy(
                                out=uT[:, c2, tt * 512 : (tt + 1) * 512], in_=ups[:, :]
                            )
        # M / xqT pools closed here

        # ======== phase 4: N2 = Wv @ Wf  [c, g] bf16 =======================
        with ExitStack() as phn:
            n2p = phn.enter_context(tc.tile_pool(name="n2p", bufs=1))
            n2 = n2p.tile([128, CC, F], BF16, tag="n2")  # 32KB/part

            with ExitStack() as ph4:
                wvp = ph4.enter_context(tc.tile_pool(name="wvp", bufs=1))
                ps4 = ph4.enter_context(tc.tile_pool(name="ps4", bufs=1, space="PSUM"))

                wvT = wvp.tile([128, FC, C], BF16, tag="wvT")  # 32KB/part
                for f in range(FC):
                    nc.sync.dma_start(
                        out=wvT[:, f, :], in_=wvT_ext[f * 128 : (f + 1) * 128, :]
                    )
                # 4 strips of 512 g-columns; Wf slices cached bf16 per strip
                for g in range(4):
                    wfb = wvp.tile([128, FC, 512], BF16, tag="wfb")  # 16KB/part
                    for f in range(FC):
                        nc.sync.dma_start(
                            out=wfb[:, f, :],
                            in_=wf_ext[
                                f * 128 : (f + 1) * 128, g * 512 : (g + 1) * 512
                            ],
                        )
                    for ah in range(2):
                        nps = ps4.tile([128, 4, 512], F32, tag="nps")  # 4 banks
                        for f in range(FC):
                            for a4 in range(4):
                                nc.tensor.matmul(
                                    nps[:, a4, :],
                                    wvT[:, f, (ah * 4 + a4) * 128 : (ah * 4 + a4 + 1) * 128],
                                    wfb[:, f, :],
                                    start=(f == 0),
                                    stop=(f == FC - 1),
                                )
                        for a4 in range(4):
                            nc.vector.tensor_copy(
                                out=n2[:, ah * 4 + a4, g * 512 : (g + 1) * 512],
                                in_=nps[:, a4, :],
                            )

            # ======== phase 5: vf = x @ N2 -> DRAM (bf16, all T rows) ======
            vf_dram = dram.tile([NB, 128, F], BF16, tag="vf_dram")
            with ExitStack() as ph5:
                st5 = ph5.enter_context(tc.tile_pool(name="st5", bufs=2))
                ps5 = ph5.enter_context(tc.tile_pool(name="ps5", bufs=2, space="PSUM"))
                for tb in range(NB):
                    xtb = st5.tile([128, CC, 128], BF16, tag="xtb")
                    for cc in range(CC):
                        nc.vector.tensor_copy(
                            out=xtb[:, cc, :], in_=xT[:, cc, tb * 128 : (tb + 1) * 128]
                        )
                    vrow = st5.tile([128, F], BF16, tag="vrow")
                    for gh in range(2):  # halves of 1024 g-columns (2 banks each)
                        vps = ps5.tile([128, 1024], F32, tag="vps")
                        for g in range(2):
                            gg = gh * 2 + g
                            for cc in range(CC):
                                nc.tensor.matmul(
                                    vps[:, g * 512 : (g + 1) * 512],
                                    xtb[:, cc, :],
                                    n2[:, cc, gg * 512 : (gg + 1) * 512],
                                    start=(cc == 0),
                                    stop=(cc == CC - 1),
                                )
                        nc.vector.tensor_copy(
                            out=vrow[:, gh * 1024 : (gh + 1) * 1024], in_=vps[:, :]
                        )
                    nc.sync.dma_start(out=vf_dram[tb], in_=vrow)
        # n2 pool closed here

        # ======== phase 6: attention over owned blocks =====================
        with ExitStack() as ph6:
            at = ph6.enter_context(tc.tile_pool(name="at", bufs=1))
            st6 = ph6.enter_context(tc.tile_pool(name="st6", bufs=2))
            vrd = ph6.enter_context(tc.tile_pool(name="vrd", bufs=4))
            small = ph6.enter_context(tc.tile_pool(name="small", bufs=4))
            ps6 = ph6.enter_context(tc.tile_pool(name="ps6", bufs=1, space="PSUM"))

            bfb = at.tile([128, F], F32, tag="bfb")
            bf_ap = bf_ext[:]
            nc.sync.dma_start(
                out=bfb,
                in_=bass.AP(
                    tensor=bf_ap.tensor,
                    offset=bf_ap.offset,
                    ap=[[0, 128]] + list(bf_ap.ap),
                ),
            )
            m2 = at.tile([128, 8, 256], F32, tag="m2")
            for k in range(8):
                nc.sync.dma_start(out=m2[:, k, :], in_=m2_ext[k])

            for k in range(8):
                ek = E[k]
                scols = ek * 128
                # scores S = uT[k].T @ xT (fp32), computed in 2-bank psum
                # halves so the next block's scores overlap this block's
                # out-matmuls; each half lands in SBUF with the causal mask
                # (last two chunks) fused into the copy
                s_sb = st6.tile([128, T], F32, tag="s_sb")
                for h0 in range(0, scols, 1024):
                    hw = min(1024, scols - h0)
                    sps = ps6.tile([128, 1024], F32, tag="sps")  # 2 banks
                    for st in range(0, hw, 512):
                        w = min(512, hw - st)
                        for c2 in range(CC):
                            nc.tensor.matmul(
                                sps[:, st : st + w],
                                uT[:, c2, k * 128 : (k + 1) * 128],
                                xT[:, c2, h0 + st : h0 + st + w],
                                start=(c2 == 0),
                                stop=(c2 == CC - 1),
                            )
                    m0 = scols - 256  # mask window start
                    plain = min(hw, max(0, m0 - h0))
                    if plain > 0:
                        nc.vector.tensor_copy(
                            out=s_sb[:, h0 : h0 + plain], in_=sps[:, :plain]
                        )
                    if plain < hw:
                        nc.vector.tensor_add(
                            s_sb[:, h0 + plain : h0 + hw],
                            sps[:, plain:hw],
                            m2[:, k, h0 + plain - m0 : h0 + hw - m0],
                        )
                negmax = small.tile([128, 1], F32, tag="negmax")
                nc.vector.tensor_reduce(
                    out=negmax,
                    in_=s_sb[:, :scols],
                    axis=mybir.AxisListType.X,
                    op=mybir.AluOpType.max,
                    negate=True,
                )
                psb = st6.tile([128, T], BF16, tag="psb")
                rsum = small.tile([128, 1], F32, tag="rsum")
                nc.scalar.activation(
                    out=psb[:, :scols],
                    in_=s_sb[:, :scols],
                    func=mybir.ActivationFunctionType.Exp,
                    bias=negmax,
                    scale=1.0,
                    accum_out=rsum,
                )
                rinv = small.tile([128, 1], F32, tag="rinv")
                nc.vector.reciprocal(out=rinv, in_=rsum)
                # transpose probabilities: PT[s-chunk] = P[:, sc].T (bf16)
                ptsb = st6.tile([128, NB, 128], BF16, tag="ptsb")
                for sc in range(ek):
                    pt = ps_t.tile([128, 128], BF16, tag="pt")
                    nc.tensor.transpose(
                        pt[:, :], psb[:, sc * 128 : (sc + 1) * 128], identbf[:, :]
                    )
                    nc.vector.tensor_copy(out=ptsb[:, sc, :], in_=pt[:, :])
                # out = P @ vf (accumulate over s-chunks), then /rowsum + bf
                ops = ps6.tile([128, F], F32, tag="ops")  # 4 banks
                for sc in range(ek):
                    vrow = vrd.tile([128, F], BF16, tag="vread")
                    nc.sync.dma_start(out=vrow, in_=vf_dram[sc])
                    for g in range(4):
                        nc.tensor.matmul(
                            ops[:, g * 512 : (g + 1) * 512],
                            ptsb[:, sc, :],
                            vrow[:, g * 512 : (g + 1) * 512],
                            start=(sc == 0),
                            stop=(sc == ek - 1),
                        )
                orow = st6.tile([128, F], F32, tag="orow")
                nc.vector.scalar_tensor_tensor(
                    out=orow,
                    in0=ops,
                    scalar=rinv,
                    in1=bfb,
                    op0=mybir.AluOpType.mult,
                    op1=mybir.AluOpType.add,
                )
                nc.sync.dma_start(out=out_ext[k], in_=orow)

    nc.finalize()
    return nc


def _get_program():
    if "nc" not in _CACHE:
        _CACHE["nc"] = _build_program()
    return _CACHE["nc"]


def _make_in_maps(x, Wq, Wk, Wv, Wf, bf):
    x = np.ascontiguousarray(x, dtype=np.float32)
    WqT = np.ascontiguousarray(np.asarray(Wq, dtype=np.float32).T)
    WkT = np.ascontiguousarray(np.asarray(Wk, dtype=np.float32).T)
    WvTb = np.ascontiguousarray(np.asarray(Wv, dtype=np.float32).T).astype(
        ml_dtypes.bfloat16
    )
    Wfb = np.ascontiguousarray(np.asarray(Wf, dtype=np.float32)).astype(
        ml_dtypes.bfloat16
    )
    bf = np.ascontiguousarray(bf, dtype=np.float32)
    in_maps = []
    for core in range(NCORES):
        b, h = core // 2, core % 2
        own = OWN_H[h]
        xb = x[b]
        xq = np.concatenate([xb[blk * 128 : (blk + 1) * 128] for blk in own], axis=0)
        mask2 = np.zeros((8, 128, 256), dtype=np.float32)
        for k, blk in enumerate(own):
            s0 = (E[k] - 2) * 128  # global key index of mask window start
            s = s0 + np.arange(256)[None, :]
            t = blk * 128 + np.arange(128)[:, None]
            mask2[k] = np.where(s <= t, 0.0, NEG).astype(np.float32)
        in_maps.append(
            {
                "xTin": np.ascontiguousarray(xb.T),
                "xqTin": np.ascontiguousarray(xq.T),
                "mask2": mask2,
                "WqT": WqT,
                "WkT": WkT,
                "WvTb": WvTb,
                "Wfb": Wfb,
                "bf": bf,
            }
        )
    return in_maps


def run_on_hw(inputs, trace=False, trace_cores=None):
    nc = _get_program()
    in_maps = _make_in_maps(**inputs)
    res = run_bass_kernel_spmd(
        nc, in_maps, list(range(NCORES)), trace=trace, trace_cores=trace_cores
    )
    out = np.empty((B, T, F), dtype=np.float32)
    for core in range(NCORES):
        b, h = core // 2, core % 2
        own = OWN_H[h]
        o = res.results[core]["out"]  # [8, 128, F]
        for k, blk in enumerate(own):
            out[b, blk * 128 : (blk + 1) * 128, :] = o[k]
    return out, res


def kernel(x, Wq, Wk, Wv, Wf, bf):
    out, _ = run_on_hw(dict(x=x, Wq=Wq, Wk=Wk, Wv=Wv, Wf=Wf, bf=bf))
    return out


# revision 12
# speedup vs baseline: 1.5947x; 1.1363x over previous
"""Trainium2 Bass kernel for nn_CausalAttention (B=4, T=2048, d_model=1024, d_ff=2048).

Sharding: 8 cores = 4 batches x 2 query-halves. Each core owns 8 query blocks
of 128 rows, paired so causal work is balanced and the per-core program is
IDENTICAL (SPMD): the k-th owned block always computes E[k] key chunks; exact
causal masking arrives as per-core input data. Host-side input marshalling
ships operands pre-transposed (and bf16-cast where allowed) so the device
spends no PE/DVE time on layout.

Input-independent weight products and the value projection are sharded across
cores and AllGathered (on-chip collectives):
  M  = Wq @ Wk.T   - each core computes a 128-col c2 slice (its WkT slice)
  N2 = Wv @ Wf     - each core computes a 256-col g slice (its Wf slice)
  vf = x @ N2      - each batch-pair core computes its T-half (its x.T half)

Per-core math:
  uT = (xq M).T             (fp32; owned query rows only)
  S  = uT.T @ x.T == q @ k.T  (fp32 scores; contraction over d_model=1024
                               instead of d_ff=2048 - half the fp32 matmul work)
  P  = softmax(S + mask)    (max-subtracted, exp on ScalarE, bf16 probs)
  out= P @ vf / rowsum + bf (psum accumulate, scaled+biased in one DVE pass)

fp32 is required through scores: softmax here is unscaled (score std ~45) and
near-one-hot; bf16/tf32 score errors flip argmaxes and corrupt whole rows.

Constraints honored: SBUF pools are a stack allocator (LIFO open/close, pool
footprint = sum of tags, reserved at open); each PSUM accumulation group must
own its 2KB bank region.
"""

import sys
from contextlib import ExitStack

for _p in ("/opt/trn_rl_repo", "/root/.axon_site/_ro/trn_rl_repo"):
    if _p not in sys.path:
        sys.path.append(_p)

import ml_dtypes
import numpy as np

import concourse.bass as bass
import concourse.mybir as mybir
import concourse.tile as tile
from concourse import bacc
from concourse.bass_utils import run_bass_kernel_spmd
from concourse.masks import make_identity

F32 = mybir.dt.float32
BF16 = mybir.dt.bfloat16

B, T, C, F = 4, 2048, 1024, 2048
NB = T // 128  # 16 query/key blocks per batch
CC = C // 128  # 8 chunks of d_model
FC = F // 128  # 16 chunks of d_ff
NCORES = 8

# k-th owned block of each half; chosen so L(OWN_H[h][k]) <= E[k] for both h
# and sum(E)=72 (ideal causal: 68). E[k] = key chunks computed for block k.
OWN_H = {
    0: [15, 12, 11, 8, 7, 4, 3, 0],
    1: [14, 13, 10, 9, 6, 5, 2, 1],
}
E = [16, 14, 12, 10, 8, 6, 4, 2]
NEG = -1.0e30

ALL8 = [list(range(8))]
PAIRS = [[0, 1], [2, 3], [4, 5], [6, 7]]

_CACHE = {}


def _build_program():
    """Trace + finalize the (single, SPMD) Bass program."""
    nc = bacc.Bacc(None)

    # all operands arrive pre-transposed / pre-cast / pre-sliced from the host
    xT_ext = nc.declare_dram_parameter("xTin", [C, T], F32, isOutput=False)
    xqT_ext = nc.declare_dram_parameter("xqTin", [C, 1024], F32, isOutput=False)
    xvT_ext = nc.declare_dram_parameter("xvTb", [C, 1024], BF16, isOutput=False)
    m2_ext = nc.declare_dram_parameter("mask2", [8, 128, 256], F32, isOutput=False)
    wqT_ext = nc.declare_dram_parameter("WqT", [F, C], F32, isOutput=False)
    wks_ext = nc.declare_dram_parameter("WkTs", [F, 128], F32, isOutput=False)
    wvT_ext = nc.declare_dram_parameter("WvTb", [F, C], BF16, isOutput=False)
    wfs_ext = nc.declare_dram_parameter("Wfs", [F, 256], BF16, isOutput=False)
    bf_ext = nc.declare_dram_parameter("bf", [F], F32, isOutput=False)
    out_ext = nc.declare_dram_parameter("out", [8, 128, F], F32, isOutput=True)

    with tile.TileContext(nc) as tc, ExitStack() as root:
        persist = root.enter_context(tc.tile_pool(name="persist", bufs=1))
        ps_t = root.enter_context(tc.tile_pool(name="ps_t", bufs=2, space="PSUM"))
        dram = root.enter_context(tc.tile_pool(name="dram", bufs=1, space="DRAM"))

        identbf = persist.tile([128, 128], BF16, tag="identbf")
        make_identity(nc, identbf[:, :])
        # long-lived operands: xT 64KB + uT 32KB per partition
        xT = persist.tile([128, CC, T], F32, tag="xT")
        for cc in range(CC):
            nc.sync.dma_start(out=xT[:, cc, :], in_=xT_ext[cc * 128 : (cc + 1) * 128, :])
        uT = persist.tile([128, CC, 1024], F32, tag="uT")

        # collective buffers (DRAM)
        msl_d = dram.tile([C, 128], F32, tag="msl_d")
        mall_d = dram.tile([NCORES * C, 128], F32, tag="mall_d", addr_space="Shared")
        n2s_d = dram.tile([C, 256], BF16, tag="n2s_d")
        n2all_d = dram.tile([NCORES * C, 256], BF16, tag="n2all_d", addr_space="Shared")
        vfs_d = dram.tile([8, 128, F], BF16, tag="vfs_d")
        vfall_d = dram.tile([NB, 128, F], BF16, tag="vfall_d")

        # ======== phase 1: M-slice = Wq @ WkT[:, my 128 cols], AllGather ===
        with ExitStack() as ph1:
            wqp = ph1.enter_context(tc.tile_pool(name="wqp", bufs=1))
            ps1 = ph1.enter_context(tc.tile_pool(name="ps1", bufs=1, space="PSUM"))
            wqT = wqp.tile([128, FC, C], F32, tag="wqT")  # 64KB/part
            for f in range(FC):
                nc.sync.dma_start(
                    out=wqT[:, f, :], in_=wqT_ext[f * 128 : (f + 1) * 128, :]
                )
            wks = wqp.tile([128, FC, 128], F32, tag="wks")  # 8KB/part
            for f in range(FC):
                nc.sync.dma_start(
                    out=wks[:, f, :], in_=wks_ext[f * 128 : (f + 1) * 128, :]
                )
            msl_sb = wqp.tile([128, CC, 128], F32, tag="msl_sb")  # 4KB/part
            for ah in range(2):  # c1-chunk halves
                mps = ps1.tile([128, 4, 512], F32, tag="mps")  # 4 banks
                for f in range(FC):
                    for a4 in range(4):
                        nc.tensor.matmul(
                            mps[:, a4, :128],
                            wqT[:, f, (ah * 4 + a4) * 128 : (ah * 4 + a4 + 1) * 128],
                            wks[:, f, :],
                            start=(f == 0),
                            stop=(f == FC - 1),
                        )
                for a4 in range(4):
                    nc.vector.tensor_copy(
                        out=msl_sb[:, ah * 4 + a4, :], in_=mps[:, a4, :128]
                    )
            for a in range(CC):
                nc.sync.dma_start(
                    out=msl_d[a * 128 : (a + 1) * 128, :], in_=msl_sb[:, a, :]
                )
            nc.gpsimd.collective_compute(
                "AllGather",
                mybir.AluOpType.bypass,
                replica_groups=ALL8,
                ins=[msl_d[:, :]],
                outs=[mall_d[:, :]],
            )

        # ======== phase 2+3: uT = (xq M).T  [c2-chunk, owned-t] fp32 =======
        with ExitStack() as ph2:
            mxp = ph2.enter_context(tc.tile_pool(name="mxp", bufs=1))
            mM = mxp.tile([128, CC, C], F32, tag="M")  # 32KB/part
            # mall_d rows: [c2-core 8][c1 8][p 128] -> mM[p, c1, c2*128:...]
            for c2 in range(CC):
                for c1 in range(CC):
                    r0 = c2 * C + c1 * 128
                    nc.sync.dma_start(
                        out=mM[:, c1, c2 * 128 : (c2 + 1) * 128],
                        in_=mall_d[r0 : r0 + 128, :],
                    )
            xqT = mxp.tile([128, CC, 1024], F32, tag="xqT")  # 32KB/part
            for cc in range(CC):
                nc.sync.dma_start(
                    out=xqT[:, cc, :], in_=xqT_ext[cc * 128 : (cc + 1) * 128, :]
                )
            with ExitStack() as ph3:
                ps3 = ph3.enter_context(tc.tile_pool(name="ps3", bufs=2, space="PSUM"))
                for c2 in range(CC):
                    for tt in range(2):  # owned-t tiles of 512
                        ups = ps3.tile([128, 512], F32, tag="ups")
                        for c1 in range(CC):
                            nc.tensor.matmul(
                                ups[:, :],
                                mM[:, c1, c2 * 128 : (c2 + 1) * 128],
                                xqT[:, c1, tt * 512 : (tt + 1) * 512],
                                start=(c1 == 0),
                                stop=(c1 == CC - 1),
                            )
                        nc.vector.tensor_copy(
                            out=uT[:, c2, tt * 512 : (tt + 1) * 512], in_=ups[:, :]
                        )
        # M / xqT pools closed here

        # ======== phase 4: N2-slice = Wv @ Wf[:, my 256 cols], AllGather ===
        with ExitStack() as ph4:
            wvp = ph4.enter_context(tc.tile_pool(name="wvp", bufs=1))
            ps4 = ph4.enter_context(tc.tile_pool(name="ps4", bufs=1, space="PSUM"))
            wvT = wvp.tile([128, FC, C], BF16, tag="wvT")  # 32KB/part
            for f in range(FC):
                nc.sync.dma_start(
                    out=wvT[:, f, :], in_=wvT_ext[f * 128 : (f + 1) * 128, :]
                )
            wfs = wvp.tile([128, FC, 256], BF16, tag="wfs")  # 8KB/part
            for f in range(FC):
                nc.sync.dma_start(
                    out=wfs[:, f, :], in_=wfs_ext[f * 128 : (f + 1) * 128, :]
                )
            n2s_sb = wvp.tile([128, CC, 256], BF16, tag="n2s_sb")  # 4KB/part
            for ah in range(2):
                nps = ps4.tile([128, 4, 512], F32, tag="nps")  # 4 banks
                for f in range(FC):
                    for a4 in range(4):
                        nc.tensor.matmul(
                            nps[:, a4, :256],
                            wvT[:, f, (ah * 4 + a4) * 128 : (ah * 4 + a4 + 1) * 128],
                            wfs[:, f, :],
                            start=(f == 0),
                            stop=(f == FC - 1),
                        )
                for a4 in range(4):
                    nc.vector.tensor_copy(
                        out=n2s_sb[:, ah * 4 + a4, :], in_=nps[:, a4, :256]
                    )
            for a in range(CC):
                nc.sync.dma_start(
                    out=n2s_d[a * 128 : (a + 1) * 128, :], in_=n2s_sb[:, a, :]
                )
            nc.gpsimd.collective_compute(
                "AllGather",
                mybir.AluOpType.bypass,
                replica_groups=ALL8,
                ins=[n2s_d[:, :]],
                outs=[n2all_d[:, :]],
            )

        # ======== phase 5: vf-half = x[my T half] @ N2 -> AllGather(pair) ==
        with ExitStack() as ph5:
            n2p = ph5.enter_context(tc.tile_pool(name="n2p", bufs=1))
            st5 = ph5.enter_context(tc.tile_pool(name="st5", bufs=2))
            ps5 = ph5.enter_context(tc.tile_pool(name="ps5", bufs=2, space="PSUM"))
            n2 = n2p.tile([128, CC, F], BF16, tag="n2")  # 32KB/part
            # n2all_d rows: [g-core 8][c 8][p 128] -> n2[p, cc, gi*256:...]
            for gi in range(NCORES):
                for cc in range(CC):
                    r0 = gi * C + cc * 128
                    nc.sync.dma_start(
                        out=n2[:, cc, gi * 256 : (gi + 1) * 256],
                        in_=n2all_d[r0 : r0 + 128, :],
                    )
            xvb = n2p.tile([128, CC, 1024], BF16, tag="xvb")  # 16KB/part
            for cc in range(CC):
                nc.sync.dma_start(
                    out=xvb[:, cc, :], in_=xvT_ext[cc * 128 : (cc + 1) * 128, :]
                )
            for tb in range(8):  # t-blocks of my half
                vrow = st5.tile([128, F], BF16, tag="vrow")
                for gh in range(2):  # halves of 1024 g-columns (2 banks each)
                    vps = ps5.tile([128, 1024], F32, tag="vps")
                    for g in range(2):
                        gg = gh * 2 + g
                        for cc in range(CC):
                            nc.tensor.matmul(
                                vps[:, g * 512 : (g + 1) * 512],
                                xvb[:, cc, tb * 128 : (tb + 1) * 128],
                                n2[:, cc, gg * 512 : (gg + 1) * 512],
                                start=(cc == 0),
                                stop=(cc == CC - 1),
                            )
                    nc.vector.tensor_copy(
                        out=vrow[:, gh * 1024 : (gh + 1) * 1024], in_=vps[:, :]
                    )
                nc.sync.dma_start(out=vfs_d[tb], in_=vrow)
            nc.gpsimd.collective_compute(
                "AllGather",
                mybir.AluOpType.bypass,
                replica_groups=PAIRS,
                ins=[vfs_d[:, :, :]],
                outs=[vfall_d[:, :, :]],
            )

        # ======== phase 6: attention over owned blocks =====================
        with ExitStack() as ph6:
            at = ph6.enter_context(tc.tile_pool(name="at", bufs=1))
            st6 = ph6.enter_context(tc.tile_pool(name="st6", bufs=2))
            vrd = ph6.enter_context(tc.tile_pool(name="vrd", bufs=4))
            small = ph6.enter_context(tc.tile_pool(name="small", bufs=4))
            ps6 = ph6.enter_context(tc.tile_pool(name="ps6", bufs=1, space="PSUM"))

            bfb = at.tile([128, F], F32, tag="bfb")
            bf_ap = bf_ext[:]
            nc.sync.dma_start(
                out=bfb,
                in_=bass.AP(
                    tensor=bf_ap.tensor,
                    offset=bf_ap.offset,
                    ap=[[0, 128]] + list(bf_ap.ap),
                ),
            )
            m2 = at.tile([128, 8, 256], F32, tag="m2")
            for k in range(8):
                nc.sync.dma_start(out=m2[:, k, :], in_=m2_ext[k])

            for k in range(8):
                ek = E[k]
                scols = ek * 128
                # scores S = uT[k].T @ xT (fp32), in 2-bank psum halves so the
                # next block's scores overlap this block's out-matmuls; each
                # half lands in SBUF with the causal mask fused into the copy
                s_sb = st6.tile([128, T], F32, tag="s_sb")
                for h0 in range(0, scols, 1024):
                    hw = min(1024, scols - h0)
                    sps = ps6.tile([128, 1024], F32, tag="sps")  # 2 banks
                    for st in range(0, hw, 512):
                        w = min(512, hw - st)
                        for c2 in range(CC):
                            nc.tensor.matmul(
                                sps[:, st : st + w],
                                uT[:, c2, k * 128 : (k + 1) * 128],
                                xT[:, c2, h0 + st : h0 + st + w],
                                start=(c2 == 0),
                                stop=(c2 == CC - 1),
                            )
                    m0 = scols - 256  # mask window start
                    plain = min(hw, max(0, m0 - h0))
                    if plain > 0:
                        nc.vector.tensor_copy(
                            out=s_sb[:, h0 : h0 + plain], in_=sps[:, :plain]
                        )
                    if plain < hw:
                        nc.vector.tensor_add(
                            s_sb[:, h0 + plain : h0 + hw],
                            sps[:, plain:hw],
                            m2[:, k, h0 + plain - m0 : h0 + hw - m0],
                        )
                negmax = small.tile([128, 1], F32, tag="negmax")
                nc.vector.tensor_reduce(
                    out=negmax,
                    in_=s_sb[:, :scols],
                    axis=mybir.AxisListType.X,
                    op=mybir.AluOpType.max,
                    negate=True,
                )
                psb = st6.tile([128, T], BF16, tag="psb")
                rsum = small.tile([128, 1], F32, tag="rsum")
                nc.scalar.activation(
                    out=psb[:, :scols],
                    in_=s_sb[:, :scols],
                    func=mybir.ActivationFunctionType.Exp,
                    bias=negmax,
                    scale=1.0,
                    accum_out=rsum,
                )
                rinv = small.tile([128, 1], F32, tag="rinv")
                nc.vector.reciprocal(out=rinv, in_=rsum)
                # transpose probabilities: PT[s-chunk] = P[:, sc].T (bf16)
                ptsb = st6.tile([128, NB, 128], BF16, tag="ptsb")
                for sc in range(ek):
                    pt = ps_t.tile([128, 128], BF16, tag="pt")
                    nc.tensor.transpose(
                        pt[:, :], psb[:, sc * 128 : (sc + 1) * 128], identbf[:, :]
                    )
                    nc.vector.tensor_copy(out=ptsb[:, sc, :], in_=pt[:, :])
                # out = P @ vf (accumulate over s-chunks), then /rowsum + bf
                ops = ps6.tile([128, F], F32, tag="ops")  # 4 banks
                for sc in range(ek):
                    vrow = vrd.tile([128, F], BF16, tag="vread")
                    nc.sync.dma_start(out=vrow, in_=vfall_d[sc])
                    for g in range(4):
                        nc.tensor.matmul(
                            ops[:, g * 512 : (g + 1) * 512],
                            ptsb[:, sc, :],
                            vrow[:, g * 512 : (g + 1) * 512],
                            start=(sc == 0),
                            stop=(sc == ek - 1),
                        )
                orow = st6.tile([128, F], F32, tag="orow")
                nc.vector.scalar_tensor_tensor(
                    out=orow,
                    in0=ops,
                    scalar=rinv,
                    in1=bfb,
                    op0=mybir.AluOpType.mult,
                    op1=mybir.AluOpType.add,
                )
                nc.sync.dma_start(out=out_ext[k], in_=orow)

    nc.finalize()
    return nc


def _get_program():
    if "nc" not in _CACHE:
        _CACHE["nc"] = _build_program()
    return _CACHE["nc"]


def _make_in_maps(x, Wq, Wk, Wv, Wf, bf):
    x = np.ascontiguousarray(x, dtype=np.float32)
    WqT = np.ascontiguousarray(np.asarray(Wq, dtype=np.float32).T)
    WkT = np.ascontiguousarray(np.asarray(Wk, dtype=np.float32).T)
    WvTb = np.ascontiguousarray(np.asarray(Wv, dtype=np.float32).T).astype(
        ml_dtypes.bfloat16
    )
    Wfb = np.asarray(Wf, dtype=np.float32).astype(ml_dtypes.bfloat16)
    bf = np.ascontiguousarray(bf, dtype=np.float32)
    in_maps = []
    for core in range(NCORES):
        b, h = core // 2, core % 2
        own = OWN_H[h]
        xb = x[b]
        xq = np.concatenate([xb[blk * 128 : (blk + 1) * 128] for blk in own], axis=0)
        mask2 = np.zeros((8, 128, 256), dtype=np.float32)
        for k, blk in enumerate(own):
            s0 = (E[k] - 2) * 128  # global key index of mask window start
            s = s0 + np.arange(256)[None, :]
            t = blk * 128 + np.arange(128)[:, None]
            mask2[k] = np.where(s <= t, 0.0, NEG).astype(np.float32)
        xvTb = (
            np.ascontiguousarray(xb[h * 1024 : (h + 1) * 1024].T)
            .astype(ml_dtypes.bfloat16)
        )
        in_maps.append(
            {
                "xTin": np.ascontiguousarray(xb.T),
                "xqTin": np.ascontiguousarray(xq.T),
                "xvTb": xvTb,
                "mask2": mask2,
                "WqT": WqT,
                "WkTs": np.ascontiguousarray(WkT[:, core * 128 : (core + 1) * 128]),
                "WvTb": WvTb,
                "Wfs": np.ascontiguousarray(Wfb[:, core * 256 : (core + 1) * 256]),
                "bf": bf,
            }
        )
    return in_maps


def run_on_hw(inputs, trace=False, trace_cores=None):
    nc = _get_program()
    in_maps = _make_in_maps(**inputs)
    res = run_bass_kernel_spmd(
        nc, in_maps, list(range(NCORES)), trace=trace, trace_cores=trace_cores
    )
    out = np.empty((B, T, F), dtype=np.float32)
    for core in range(NCORES):
        b, h = core // 2, core % 2
        own = OWN_H[h]
        o = res.results[core]["out"]  # [8, 128, F]
        for k, blk in enumerate(own):
            out[b, blk * 128 : (blk + 1) * 128, :] = o[k]
    return out, res


def kernel(x, Wq, Wk, Wv, Wf, bf):
    out, _ = run_on_hw(dict(x=x, Wq=Wq, Wk=Wk, Wv=Wv, Wf=Wf, bf=bf))
    return out


# revision 15
# speedup vs baseline: 1.8269x; 1.1456x over previous
"""Trainium2 Bass kernel for nn_CausalAttention (B=4, T=2048, d_model=1024, d_ff=2048).

Sharding: 8 cores = 4 batches x 2 query-halves. Each core owns 8 query blocks
of 128 rows, paired so causal work is balanced and the per-core program is
IDENTICAL (SPMD): the k-th owned block always computes E[k] key chunks; exact
causal masking arrives as per-core input data. Host-side input marshalling
ships operands pre-transposed (and bf16-cast where allowed) so the device
spends no PE/DVE time on layout.

Input-independent weight products and the value projection are sharded across
cores and AllGathered (on-chip collectives), with independent matmul phases
ordered to hide each gather's latency:
  M  = Wq @ Wk.T   - each core computes a 128-col c2 slice (its WkT slice)
  N2 = Wv @ Wf     - each core computes a 256-col g slice (its Wf slice)
  vf = x @ N2      - each batch-pair core computes its T-half (its x.T half)

Per-core math:
  uT = (xq M).T             (fp32; owned query rows only)
  S  = uT.T @ x.T == q @ k.T  (fp32 scores; contraction over d_model=1024
                               instead of d_ff=2048 - half the fp32 matmul work)
  P  = softmax(S + mask)    (max-subtracted, exp on ScalarE, bf16 probs)
  out= P @ vf / rowsum + bf (psum accumulate, scaled+biased in one DVE pass)

fp32 is required through scores: softmax here is unscaled (score std ~45) and
near-one-hot; bf16/tf32 score errors flip argmaxes and corrupt whole rows.

Constraints honored: SBUF pools are a stack allocator (LIFO open/close, pool
footprint = sum of tags, reserved at open); each PSUM accumulation group must
own its 2KB bank region; per-engine instruction streams execute in order, so
emission order is used to cover collective/DMA waits with independent work.
"""

import sys
from contextlib import ExitStack

for _p in ("/opt/trn_rl_repo", "/root/.axon_site/_ro/trn_rl_repo"):
    if _p not in sys.path:
        sys.path.append(_p)

import ml_dtypes
import numpy as np

import concourse.bass as bass
import concourse.mybir as mybir
import concourse.tile as tile
from concourse import bacc
from concourse.bass_utils import run_bass_kernel_spmd
from concourse.masks import make_identity

F32 = mybir.dt.float32
BF16 = mybir.dt.bfloat16

B, T, C, F = 4, 2048, 1024, 2048
NB = T // 128  # 16 query/key blocks per batch
CC = C // 128  # 8 chunks of d_model
FC = F // 128  # 16 chunks of d_ff
NCORES = 8

# k-th owned block of each half; chosen so L(OWN_H[h][k]) <= E[k] for both h
# and sum(E)=72 (ideal causal: 68). E[k] = key chunks computed for block k.
OWN_H = {
    0: [15, 12, 11, 8, 7, 4, 3, 0],
    1: [14, 13, 10, 9, 6, 5, 2, 1],
}
E = [16, 14, 12, 10, 8, 6, 4, 2]
NEG = -1.0e30

ALL8 = [list(range(8))]
PAIRS = [[0, 1], [2, 3], [4, 5], [6, 7]]

_CACHE = {}


def _build_program():
    """Trace + finalize the (single, SPMD) Bass program."""
    nc = bacc.Bacc(None)

    # all operands arrive pre-transposed / pre-cast / pre-sliced from the host
    xT_ext = nc.declare_dram_parameter("xTin", [C, T], F32, isOutput=False)
    xqT_ext = nc.declare_dram_parameter("xqTin", [C, 1024], F32, isOutput=False)
    xvT_ext = nc.declare_dram_parameter("xvTb", [C, 1024], BF16, isOutput=False)
    m2_ext = nc.declare_dram_parameter("mask2", [8, 128, 256], F32, isOutput=False)
    wqT_ext = nc.declare_dram_parameter("WqT", [F, C], F32, isOutput=False)
    wks_ext = nc.declare_dram_parameter("WkTs", [F, 128], F32, isOutput=False)
    wvT_ext = nc.declare_dram_parameter("WvTb", [F, C], BF16, isOutput=False)
    wfs_ext = nc.declare_dram_parameter("Wfs", [F, 256], BF16, isOutput=False)
    bf_ext = nc.declare_dram_parameter("bf", [F], F32, isOutput=False)
    out_ext = nc.declare_dram_parameter("out", [8, 128, F], F32, isOutput=True)

    with tile.TileContext(nc) as tc, ExitStack() as root:
        persist = root.enter_context(tc.tile_pool(name="persist", bufs=1))
        ps_t = root.enter_context(tc.tile_pool(name="ps_t", bufs=2, space="PSUM"))
        dram = root.enter_context(tc.tile_pool(name="dram", bufs=1, space="DRAM"))

        identbf = persist.tile([128, 128], BF16, tag="identbf")
        make_identity(nc, identbf[:, :])
        # long-lived operands (loads emitted late, where first needed)
        xT = persist.tile([128, CC, T], F32, tag="xT")  # 64KB/part
        uT = persist.tile([128, CC, 1024], F32, tag="uT")  # 32KB/part

        # collective buffers (DRAM)
        msl_d = dram.tile([C, 128], F32, tag="msl_d")
        mall_d = dram.tile([NCORES * C, 128], F32, tag="mall_d", addr_space="Shared")
        n2s_d = dram.tile([C, 256], BF16, tag="n2s_d")
        n2all_d = dram.tile([NCORES * C, 256], BF16, tag="n2all_d", addr_space="Shared")
        vfs_d = dram.tile([8, 128, F], BF16, tag="vfs_d")
        vfall_d = dram.tile([NB, 128, F], BF16, tag="vfall_d")

        # ======== phase 1: M-slice = Wq @ WkT[:, my 128 cols], AllGather ===
        with ExitStack() as ph1:
            wqp = ph1.enter_context(tc.tile_pool(name="wqp", bufs=1))
            ps1 = ph1.enter_context(tc.tile_pool(name="ps1", bufs=1, space="PSUM"))
            wqT = wqp.tile([128, FC, C], F32, tag="wqT")  # 64KB/part
            wks = wqp.tile([128, FC, 128], F32, tag="wks")  # 8KB/part
            for f in range(FC):  # interleaved so f=0 operands arrive first
                nc.sync.dma_start(
                    out=wks[:, f, :], in_=wks_ext[f * 128 : (f + 1) * 128, :]
                )
                nc.sync.dma_start(
                    out=wqT[:, f, :], in_=wqT_ext[f * 128 : (f + 1) * 128, :]
                )
            msl_sb = wqp.tile([128, CC, 128], F32, tag="msl_sb")  # 4KB/part
            for ah in range(2):  # c1-chunk halves
                mps = ps1.tile([128, 4, 512], F32, tag="mps")  # 4 banks
                for f in range(FC):
                    for a4 in range(4):
                        nc.tensor.matmul(
                            mps[:, a4, :128],
                            wqT[:, f, (ah * 4 + a4) * 128 : (ah * 4 + a4 + 1) * 128],
                            wks[:, f, :],
                            start=(f == 0),
                            stop=(f == FC - 1),
                        )
                for a4 in range(4):
                    nc.vector.tensor_copy(
                        out=msl_sb[:, ah * 4 + a4, :], in_=mps[:, a4, :128]
                    )
            for a in range(CC):
                nc.sync.dma_start(
                    out=msl_d[a * 128 : (a + 1) * 128, :], in_=msl_sb[:, a, :]
                )
            nc.gpsimd.collective_compute(
                "AllGather",
                mybir.AluOpType.bypass,
                replica_groups=ALL8,
                ins=[msl_d[:, :]],
                outs=[mall_d[:, :]],
            )

        # ======== phase 2: N2-slice = Wv @ Wf[:, my 256 cols], AllGather ===
        # (runs on PE while the M AllGather is in flight)
        with ExitStack() as ph4:
            wvp = ph4.enter_context(tc.tile_pool(name="wvp", bufs=1))
            ps4 = ph4.enter_context(tc.tile_pool(name="ps4", bufs=1, space="PSUM"))
            wvT = wvp.tile([128, FC, C], BF16, tag="wvT")  # 32KB/part
            wfs = wvp.tile([128, FC, 256], BF16, tag="wfs")  # 8KB/part
            for f in range(FC):
                nc.sync.dma_start(
                    out=wfs[:, f, :], in_=wfs_ext[f * 128 : (f + 1) * 128, :]
                )
                nc.sync.dma_start(
                    out=wvT[:, f, :], in_=wvT_ext[f * 128 : (f + 1) * 128, :]
                )
            n2s_sb = wvp.tile([128, CC, 256], BF16, tag="n2s_sb")  # 4KB/part
            for ah in range(2):
                nps = ps4.tile([128, 4, 512], F32, tag="nps")  # 4 banks
                for f in range(FC):
                    for a4 in range(4):
                        nc.tensor.matmul(
                            nps[:, a4, :256],
                            wvT[:, f, (ah * 4 + a4) * 128 : (ah * 4 + a4 + 1) * 128],
                            wfs[:, f, :],
                            start=(f == 0),
                            stop=(f == FC - 1),
                        )
                for a4 in range(4):
                    nc.vector.tensor_copy(
                        out=n2s_sb[:, ah * 4 + a4, :], in_=nps[:, a4, :256]
                    )
            for a in range(CC):
                nc.sync.dma_start(
                    out=n2s_d[a * 128 : (a + 1) * 128, :], in_=n2s_sb[:, a, :]
                )
            nc.gpsimd.collective_compute(
                "AllGather",
                mybir.AluOpType.bypass,
                replica_groups=ALL8,
                ins=[n2s_d[:, :]],
                outs=[n2all_d[:, :]],
            )

        # ======== phase 3: uT = (xq M).T  [c2-chunk, owned-t] fp32 =========
        with ExitStack() as ph2:
            mxp = ph2.enter_context(tc.tile_pool(name="mxp", bufs=1))
            mM = mxp.tile([128, CC, C], F32, tag="M")  # 32KB/part
            xqT = mxp.tile([128, CC, 1024], F32, tag="xqT")  # 32KB/part
            for cc in range(CC):
                nc.sync.dma_start(
                    out=xqT[:, cc, :], in_=xqT_ext[cc * 128 : (cc + 1) * 128, :]
                )
            # mall_d rows: [c2-core 8][c1 8][p 128] -> mM[p, c1, c2*128:...]
            for c2 in range(CC):
                for c1 in range(CC):
                    r0 = c2 * C + c1 * 128
                    nc.sync.dma_start(
                        out=mM[:, c1, c2 * 128 : (c2 + 1) * 128],
                        in_=mall_d[r0 : r0 + 128, :],
                    )
            with ExitStack() as ph3:
                ps3 = ph3.enter_context(tc.tile_pool(name="ps3", bufs=2, space="PSUM"))
                for c2 in range(CC):
                    for tt in range(2):  # owned-t tiles of 512
                        ups = ps3.tile([128, 512], F32, tag="ups")
                        for c1 in range(CC):
                            nc.tensor.matmul(
                                ups[:, :],
                                mM[:, c1, c2 * 128 : (c2 + 1) * 128],
                                xqT[:, c1, tt * 512 : (tt + 1) * 512],
                                start=(c1 == 0),
                                stop=(c1 == CC - 1),
                            )
                        nc.vector.tensor_copy(
                            out=uT[:, c2, tt * 512 : (tt + 1) * 512], in_=ups[:, :]
                        )
        # M / xqT pools closed here

        # ======== phase 4: vf-half = x[my T half] @ N2 -> AllGather(pair) ==
        with ExitStack() as ph5:
            n2p = ph5.enter_context(tc.tile_pool(name="n2p", bufs=1))
            st5 = ph5.enter_context(tc.tile_pool(name="st5", bufs=2))
            ps5 = ph5.enter_context(tc.tile_pool(name="ps5", bufs=2, space="PSUM"))
            n2 = n2p.tile([128, CC, F], BF16, tag="n2")  # 32KB/part
            xvb = n2p.tile([128, CC, 1024], BF16, tag="xvb")  # 16KB/part
            for cc in range(CC):
                nc.sync.dma_start(
                    out=xvb[:, cc, :], in_=xvT_ext[cc * 128 : (cc + 1) * 128, :]
                )
            # n2all_d rows: [g-core 8][c 8][p 128] -> n2[p, cc, gi*256:...]
            for gi in range(NCORES):
                for cc in range(CC):
                    r0 = gi * C + cc * 128
                    nc.sync.dma_start(
                        out=n2[:, cc, gi * 256 : (gi + 1) * 256],
                        in_=n2all_d[r0 : r0 + 128, :],
                    )
            for tb in range(8):  # t-blocks of my half
                vrow = st5.tile([128, F], BF16, tag="vrow")
                for gh in range(2):  # halves of 1024 g-columns (2 banks each)
                    vps = ps5.tile([128, 1024], F32, tag="vps")
                    for g in range(2):
                        gg = gh * 2 + g
                        for cc in range(CC):
                            nc.tensor.matmul(
                                vps[:, g * 512 : (g + 1) * 512],
                                xvb[:, cc, tb * 128 : (tb + 1) * 128],
                                n2[:, cc, gg * 512 : (gg + 1) * 512],
                                start=(cc == 0),
                                stop=(cc == CC - 1),
                            )
                    nc.vector.tensor_copy(
                        out=vrow[:, gh * 1024 : (gh + 1) * 1024], in_=vps[:, :]
                    )
                nc.sync.dma_start(out=vfs_d[tb], in_=vrow)
            nc.gpsimd.collective_compute(
                "AllGather",
                mybir.AluOpType.bypass,
                replica_groups=PAIRS,
                ins=[vfs_d[:, :, :]],
                outs=[vfall_d[:, :, :]],
            )
            # xT loads here: overlap the vf matmuls / AllGather; first needed
            # by the attention score matmuls
            for cc in range(CC):
                nc.sync.dma_start(
                    out=xT[:, cc, :], in_=xT_ext[cc * 128 : (cc + 1) * 128, :]
                )

        # ======== phase 5: attention over owned blocks =====================
        # software-pipelined: block k's scores/softmax run before block k-1's
        # out-matmuls are retired, covering the vf AllGather and the per-block
        # softmax latency with PE work
        with ExitStack() as ph6:
            at = ph6.enter_context(tc.tile_pool(name="at", bufs=1))
            st6 = ph6.enter_context(tc.tile_pool(name="st6", bufs=2))
            vrd = ph6.enter_context(tc.tile_pool(name="vrd", bufs=4))
            small = ph6.enter_context(tc.tile_pool(name="small", bufs=4))
            ps6 = ph6.enter_context(tc.tile_pool(name="ps6", bufs=1, space="PSUM"))

            bfb = at.tile([128, F], F32, tag="bfb")
            bf_ap = bf_ext[:]
            nc.sync.dma_start(
                out=bfb,
                in_=bass.AP(
                    tensor=bf_ap.tensor,
                    offset=bf_ap.offset,
                    ap=[[0, 128]] + list(bf_ap.ap),
                ),
            )
            m2 = at.tile([128, 8, 256], F32, tag="m2")
            for k in range(8):
                nc.sync.dma_start(out=m2[:, k, :], in_=m2_ext[k])

            def softmax_stage(k):
                """scores -> masked SBUF copy -> exp -> transposed bf16 probs"""
                ek = E[k]
                scols = ek * 128
                s_sb = st6.tile([128, T], F32, tag="s_sb", name=f"s_sb{k}")
                for h0 in range(0, scols, 1024):
                    hw = min(1024, scols - h0)
                    sps = ps6.tile([128, 1024], F32, tag="sps", name=f"sps{k}_{h0}")
                    for st in range(0, hw, 512):
                        w = min(512, hw - st)
                        for c2 in range(CC):
                            nc.tensor.matmul(
                                sps[:, st : st + w],
                                uT[:, c2, k * 128 : (k + 1) * 128],
                                xT[:, c2, h0 + st : h0 + st + w],
                                start=(c2 == 0),
                                stop=(c2 == CC - 1),
                            )
                    m0 = scols - 256  # mask window start
                    plain = min(hw, max(0, m0 - h0))
                    if plain > 0:
                        nc.vector.tensor_copy(
                            out=s_sb[:, h0 : h0 + plain], in_=sps[:, :plain]
                        )
                    if plain < hw:
                        nc.vector.tensor_add(
                            s_sb[:, h0 + plain : h0 + hw],
                            sps[:, plain:hw],
                            m2[:, k, h0 + plain - m0 : h0 + hw - m0],
                        )
                negmax = small.tile([128, 1], F32, tag="negmax", name=f"negmax{k}")
                nc.vector.tensor_reduce(
                    out=negmax,
                    in_=s_sb[:, :scols],
                    axis=mybir.AxisListType.X,
                    op=mybir.AluOpType.max,
                    negate=True,
                )
                psb = st6.tile([128, T], BF16, tag="psb", name=f"psb{k}", bufs=3)
                rsum = small.tile([128, 1], F32, tag="rsum", name=f"rsum{k}")
                nc.scalar.activation(
                    out=psb[:, :scols],
                    in_=s_sb[:, :scols],
                    func=mybir.ActivationFunctionType.Exp,
                    bias=negmax,
                    scale=1.0,
                    accum_out=rsum,
                )
                rinv = small.tile([128, 1], F32, tag="rinv", name=f"rinv{k}")
                nc.vector.reciprocal(out=rinv, in_=rsum)
                return psb, rinv

            def out_stage(k, psb, rinv):
                """transpose probs, out = P @ vf (accum over s-chunks), epilogue"""
                ek = E[k]
                ptsb = st6.tile([128, NB, 128], BF16, tag="ptsb", name=f"ptsb{k}")
                for sc in range(ek):
                    pt = ps_t.tile([128, 128], BF16, tag="pt", name=f"pt{k}_{sc}")
                    nc.tensor.transpose(
                        pt[:, :], psb[:, sc * 128 : (sc + 1) * 128], identbf[:, :]
                    )
                    nc.vector.tensor_copy(out=ptsb[:, sc, :], in_=pt[:, :])
                ops = ps6.tile([128, F], F32, tag="ops", name=f"ops{k}")  # 4 banks
                for sc in range(ek):
                    vrow = vrd.tile([128, F], BF16, tag="vread", name=f"vread{k}_{sc}")
                    nc.sync.dma_start(out=vrow, in_=vfall_d[sc])
                    for g in range(4):
                        nc.tensor.matmul(
                            ops[:, g * 512 : (g + 1) * 512],
                            ptsb[:, sc, :],
                            vrow[:, g * 512 : (g + 1) * 512],
                            start=(sc == 0),
                            stop=(sc == ek - 1),
                        )
                orow = st6.tile([128, F], F32, tag="orow", name=f"orow{k}")
                nc.vector.scalar_tensor_tensor(
                    out=orow,
                    in0=ops,
                    scalar=rinv,
                    in1=bfb,
                    op0=mybir.AluOpType.mult,
                    op1=mybir.AluOpType.add,
                )
                nc.sync.dma_start(out=out_ext[k], in_=orow)

            pending = None
            for k in range(8):
                staged = softmax_stage(k)
                if pending is not None:
                    out_stage(k - 1, *pending)
                pending = staged
            out_stage(7, *pending)

    nc.finalize()
    return nc


def _get_program():
    if "nc" not in _CACHE:
        _CACHE["nc"] = _build_program()
    return _CACHE["nc"]


def _make_in_maps(x, Wq, Wk, Wv, Wf, bf):
    x = np.ascontiguousarray(x, dtype=np.float32)
    WqT = np.ascontiguousarray(np.asarray(Wq, dtype=np.float32).T)
    WkT = np.ascontiguousarray(np.asarray(Wk, dtype=np.float32).T)
    WvTb = np.ascontiguousarray(np.asarray(Wv, dtype=np.float32).T).astype(
        ml_dtypes.bfloat16
    )
    Wfb = np.asarray(Wf, dtype=np.float32).astype(ml_dtypes.bfloat16)
    bf = np.ascontiguousarray(bf, dtype=np.float32)
    in_maps = []
    for core in range(NCORES):
        b, h = core // 2, core % 2
        own = OWN_H[h]
        xb = x[b]
        xq = np.concatenate([xb[blk * 128 : (blk + 1) * 128] for blk in own], axis=0)
        mask2 = np.zeros((8, 128, 256), dtype=np.float32)
        for k, blk in enumerate(own):
            s0 = (E[k] - 2) * 128  # global key index of mask window start
            s = s0 + np.arange(256)[None, :]
            t = blk * 128 + np.arange(128)[:, None]
            mask2[k] = np.where(s <= t, 0.0, NEG).astype(np.float32)
        xvTb = (
            np.ascontiguousarray(xb[h * 1024 : (h + 1) * 1024].T)
            .astype(ml_dtypes.bfloat16)
        )
        in_maps.append(
            {
                "xTin": np.ascontiguousarray(xb.T),
                "xqTin": np.ascontiguousarray(xq.T),
                "xvTb": xvTb,
                "mask2": mask2,
                "WqT": WqT,
                "WkTs": np.ascontiguousarray(WkT[:, core * 128 : (core + 1) * 128]),
                "WvTb": WvTb,
                "Wfs": np.ascontiguousarray(Wfb[:, core * 256 : (core + 1) * 256]),
                "bf": bf,
            }
        )
    return in_maps


def run_on_hw(inputs, trace=False, trace_cores=None):
    nc = _get_program()
    in_maps = _make_in_maps(**inputs)
    res = run_bass_kernel_spmd(
        nc, in_maps, list(range(NCORES)), trace=trace, trace_cores=trace_cores
    )
    out = np.empty((B, T, F), dtype=np.float32)
    for core in range(NCORES):
        b, h = core // 2, core % 2
        own = OWN_H[h]
        o = res.results[core]["out"]  # [8, 128, F]
        for k, blk in enumerate(own):
            out[b, blk * 128 : (blk + 1) * 128, :] = o[k]
    return out, res


def kernel(x, Wq, Wk, Wv, Wf, bf):
    out, _ = run_on_hw(dict(x=x, Wq=Wq, Wk=Wk, Wv=Wv, Wf=Wf, bf=bf))
    return out


# revision 16
# speedup vs baseline: 1.9175x; 1.0496x over previous
"""Trainium2 Bass kernel for nn_CausalAttention (B=4, T=2048, d_model=1024, d_ff=2048).

Sharding: 8 cores = 4 batches x 2 query-halves. Each core owns 8 query blocks
of 128 rows, paired so causal work is balanced and the per-core program is
IDENTICAL (SPMD): the k-th owned block always computes E[k] key chunks; exact
causal masking arrives as per-core input data. Host-side input marshalling
ships operands pre-transposed (and bf16-cast where allowed) so the device
spends no PE/DVE time on layout.

Input-independent weight products and the value projection are sharded across
cores and AllGathered (on-chip collectives), with independent matmul phases
ordered to hide each gather's latency:
  M  = Wq @ Wk.T   - each core computes a 128-col c2 slice (its WkT slice)
  N2 = Wv @ Wf     - each core computes a 256-col g slice (its Wf slice)
  vf = x @ N2      - each batch-pair core computes its T-half (its x.T half)

Per-core math:
  uT = (xq M).T             (fp32; owned query rows only)
  S  = uT.T @ x.T == q @ k.T  (fp32 scores; contraction over d_model=1024
                               instead of d_ff=2048 - half the fp32 matmul work)
  P  = softmax(S + mask)    (max-subtracted, exp on ScalarE, bf16 probs)
  out= P @ vf / rowsum + bf (psum accumulate, scaled+biased in one DVE pass)

fp32 is required through scores: softmax here is unscaled (score std ~45) and
near-one-hot; bf16/tf32 score errors flip argmaxes and corrupt whole rows.

Constraints honored: SBUF pools are a stack allocator (LIFO open/close, pool
footprint = sum of tags, reserved at open); each PSUM accumulation group must
own its 2KB bank region; per-engine instruction streams execute in order, so
emission order is used to cover collective/DMA waits with independent work.
"""

import sys
from contextlib import ExitStack

for _p in ("/opt/trn_rl_repo", "/root/.axon_site/_ro/trn_rl_repo"):
    if _p not in sys.path:
        sys.path.append(_p)

import ml_dtypes
import numpy as np

import concourse.bass as bass
import concourse.mybir as mybir
import concourse.tile as tile
from concourse import bacc
from concourse.bass_utils import run_bass_kernel_spmd
from concourse.masks import make_identity

F32 = mybir.dt.float32
BF16 = mybir.dt.bfloat16

B, T, C, F = 4, 2048, 1024, 2048
NB = T // 128  # 16 query/key blocks per batch
CC = C // 128  # 8 chunks of d_model
FC = F // 128  # 16 chunks of d_ff
NCORES = 8

# k-th owned block of each half; chosen so L(OWN_H[h][k]) <= E[k] for both h
# and sum(E)=72 (ideal causal: 68). E[k] = key chunks computed for block k.
OWN_H = {
    0: [15, 12, 11, 8, 7, 4, 3, 0],
    1: [14, 13, 10, 9, 6, 5, 2, 1],
}
E = [16, 14, 12, 10, 8, 6, 4, 2]
NEG = -1.0e30

ALL8 = [list(range(8))]
PAIRS = [[0, 1], [2, 3], [4, 5], [6, 7]]

_CACHE = {}


def _build_program():
    """Trace + finalize the (single, SPMD) Bass program."""
    nc = bacc.Bacc(None)

    # all operands arrive pre-transposed / pre-cast / pre-sliced from the host
    xT_ext = nc.declare_dram_parameter("xTin", [C, T], F32, isOutput=False)
    xqT_ext = nc.declare_dram_parameter("xqTin", [C, 1024], F32, isOutput=False)
    xvT_ext = nc.declare_dram_parameter("xvTb", [C, 1024], BF16, isOutput=False)
    m2_ext = nc.declare_dram_parameter("mask2", [8, 128, 256], F32, isOutput=False)
    wqT_ext = nc.declare_dram_parameter("WqT", [F, C], F32, isOutput=False)
    wks_ext = nc.declare_dram_parameter("WkTs", [F, 128], F32, isOutput=False)
    wvT_ext = nc.declare_dram_parameter("WvTb", [F, C], BF16, isOutput=False)
    wfs_ext = nc.declare_dram_parameter("Wfs", [F, 256], BF16, isOutput=False)
    bf_ext = nc.declare_dram_parameter("bf", [F], F32, isOutput=False)
    out_ext = nc.declare_dram_parameter("out", [8, 128, F], F32, isOutput=True)

    with tile.TileContext(nc) as tc, ExitStack() as root:
        persist = root.enter_context(tc.tile_pool(name="persist", bufs=1))
        ps_t = root.enter_context(tc.tile_pool(name="ps_t", bufs=2, space="PSUM"))
        dram = root.enter_context(tc.tile_pool(name="dram", bufs=1, space="DRAM"))

        identbf = persist.tile([128, 128], BF16, tag="identbf")
        make_identity(nc, identbf[:, :])
        # long-lived operands (loads emitted late, where first needed)
        xT = persist.tile([128, CC, T], F32, tag="xT")  # 64KB/part
        uT = persist.tile([128, CC, 1024], F32, tag="uT")  # 32KB/part

        # collective buffers (DRAM)
        msl_d = dram.tile([C, 128], F32, tag="msl_d")
        mall_d = dram.tile([NCORES * C, 128], F32, tag="mall_d", addr_space="Shared")
        n2s_d = dram.tile([C, 256], BF16, tag="n2s_d")
        n2all_d = dram.tile([NCORES * C, 256], BF16, tag="n2all_d", addr_space="Shared")
        vfs_d = dram.tile([8, 128, F], BF16, tag="vfs_d")
        vfallA_d = dram.tile([8, 128, F], BF16, tag="vfallA_d")
        vfallB_d = dram.tile([8, 128, F], BF16, tag="vfallB_d")

        # ======== phase 1: M-slice = Wq @ WkT[:, my 128 cols], AllGather ===
        with ExitStack() as ph1:
            wqp = ph1.enter_context(tc.tile_pool(name="wqp", bufs=1))
            ps1 = ph1.enter_context(tc.tile_pool(name="ps1", bufs=1, space="PSUM"))
            wqT = wqp.tile([128, FC, C], F32, tag="wqT")  # 64KB/part
            wks = wqp.tile([128, FC, 128], F32, tag="wks")  # 8KB/part
            for f in range(FC):  # interleaved so f=0 operands arrive first
                nc.sync.dma_start(
                    out=wks[:, f, :], in_=wks_ext[f * 128 : (f + 1) * 128, :]
                )
                nc.sync.dma_start(
                    out=wqT[:, f, :], in_=wqT_ext[f * 128 : (f + 1) * 128, :]
                )
            msl_sb = wqp.tile([128, CC, 128], F32, tag="msl_sb")  # 4KB/part
            for ah in range(2):  # c1-chunk halves
                mps = ps1.tile([128, 4, 512], F32, tag="mps")  # 4 banks
                for f in range(FC):
                    for a4 in range(4):
                        nc.tensor.matmul(
                            mps[:, a4, :128],
                            wqT[:, f, (ah * 4 + a4) * 128 : (ah * 4 + a4 + 1) * 128],
                            wks[:, f, :],
                            start=(f == 0),
                            stop=(f == FC - 1),
                        )
                for a4 in range(4):
                    nc.vector.tensor_copy(
                        out=msl_sb[:, ah * 4 + a4, :], in_=mps[:, a4, :128]
                    )
            for a in range(CC):
                nc.sync.dma_start(
                    out=msl_d[a * 128 : (a + 1) * 128, :], in_=msl_sb[:, a, :]
                )
            nc.gpsimd.collective_compute(
                "AllGather",
                mybir.AluOpType.bypass,
                replica_groups=ALL8,
                ins=[msl_d[:, :]],
                outs=[mall_d[:, :]],
            )

        # ======== phase 2: N2-slice = Wv @ Wf[:, my 256 cols], AllGather ===
        # (runs on PE while the M AllGather is in flight)
        with ExitStack() as ph4:
            wvp = ph4.enter_context(tc.tile_pool(name="wvp", bufs=1))
            ps4 = ph4.enter_context(tc.tile_pool(name="ps4", bufs=1, space="PSUM"))
            wvT = wvp.tile([128, FC, C], BF16, tag="wvT")  # 32KB/part
            wfs = wvp.tile([128, FC, 256], BF16, tag="wfs")  # 8KB/part
            for f in range(FC):
                nc.sync.dma_start(
                    out=wfs[:, f, :], in_=wfs_ext[f * 128 : (f + 1) * 128, :]
                )
                nc.sync.dma_start(
                    out=wvT[:, f, :], in_=wvT_ext[f * 128 : (f + 1) * 128, :]
                )
            n2s_sb = wvp.tile([128, CC, 256], BF16, tag="n2s_sb")  # 4KB/part
            for ah in range(2):
                nps = ps4.tile([128, 4, 512], F32, tag="nps")  # 4 banks
                for f in range(FC):
                    for a4 in range(4):
                        nc.tensor.matmul(
                            nps[:, a4, :256],
                            wvT[:, f, (ah * 4 + a4) * 128 : (ah * 4 + a4 + 1) * 128],
                            wfs[:, f, :],
                            start=(f == 0),
                            stop=(f == FC - 1),
                        )
                for a4 in range(4):
                    nc.vector.tensor_copy(
                        out=n2s_sb[:, ah * 4 + a4, :], in_=nps[:, a4, :256]
                    )
            for a in range(CC):
                nc.sync.dma_start(
                    out=n2s_d[a * 128 : (a + 1) * 128, :], in_=n2s_sb[:, a, :]
                )
            nc.gpsimd.collective_compute(
                "AllGather",
                mybir.AluOpType.bypass,
                replica_groups=ALL8,
                ins=[n2s_d[:, :]],
                outs=[n2all_d[:, :]],
            )

        # ======== phase 3: uT = (xq M).T  [c2-chunk, owned-t] fp32 =========
        with ExitStack() as ph2:
            mxp = ph2.enter_context(tc.tile_pool(name="mxp", bufs=1))
            mM = mxp.tile([128, CC, C], F32, tag="M")  # 32KB/part
            xqT = mxp.tile([128, CC, 1024], F32, tag="xqT")  # 32KB/part
            for cc in range(CC):
                nc.sync.dma_start(
                    out=xqT[:, cc, :], in_=xqT_ext[cc * 128 : (cc + 1) * 128, :]
                )
            # mall_d rows: [c2-core 8][c1 8][p 128] -> mM[p, c1, c2*128:...]
            for c2 in range(CC):
                for c1 in range(CC):
                    r0 = c2 * C + c1 * 128
                    nc.sync.dma_start(
                        out=mM[:, c1, c2 * 128 : (c2 + 1) * 128],
                        in_=mall_d[r0 : r0 + 128, :],
                    )
            with ExitStack() as ph3:
                ps3 = ph3.enter_context(tc.tile_pool(name="ps3", bufs=2, space="PSUM"))
                for c2 in range(CC):
                    for tt in range(2):  # owned-t tiles of 512
                        ups = ps3.tile([128, 512], F32, tag="ups")
                        for c1 in range(CC):
                            nc.tensor.matmul(
                                ups[:, :],
                                mM[:, c1, c2 * 128 : (c2 + 1) * 128],
                                xqT[:, c1, tt * 512 : (tt + 1) * 512],
                                start=(c1 == 0),
                                stop=(c1 == CC - 1),
                            )
                        nc.vector.tensor_copy(
                            out=uT[:, c2, tt * 512 : (tt + 1) * 512], in_=ups[:, :]
                        )
        # M / xqT pools closed here

        # ======== phase 4: vf-half = x[my T half] @ N2 -> AllGather(pair) ==
        with ExitStack() as ph5:
            n2p = ph5.enter_context(tc.tile_pool(name="n2p", bufs=1))
            st5 = ph5.enter_context(tc.tile_pool(name="st5", bufs=2))
            ps5 = ph5.enter_context(tc.tile_pool(name="ps5", bufs=2, space="PSUM"))
            n2 = n2p.tile([128, CC, F], BF16, tag="n2")  # 32KB/part
            xvb = n2p.tile([128, CC, 1024], BF16, tag="xvb")  # 16KB/part
            for cc in range(CC):
                nc.sync.dma_start(
                    out=xvb[:, cc, :], in_=xvT_ext[cc * 128 : (cc + 1) * 128, :]
                )
            # n2all_d rows: [g-core 8][c 8][p 128] -> n2[p, cc, gi*256:...]
            for gi in range(NCORES):
                for cc in range(CC):
                    r0 = gi * C + cc * 128
                    nc.sync.dma_start(
                        out=n2[:, cc, gi * 256 : (gi + 1) * 256],
                        in_=n2all_d[r0 : r0 + 128, :],
                    )
            for tb in range(8):  # t-blocks of my half
                vrow = st5.tile([128, F], BF16, tag="vrow")
                for gh in range(2):  # halves of 1024 g-columns (2 banks each)
                    vps = ps5.tile([128, 1024], F32, tag="vps")
                    for g in range(2):
                        gg = gh * 2 + g
                        for cc in range(CC):
                            nc.tensor.matmul(
                                vps[:, g * 512 : (g + 1) * 512],
                                xvb[:, cc, tb * 128 : (tb + 1) * 128],
                                n2[:, cc, gg * 512 : (gg + 1) * 512],
                                start=(cc == 0),
                                stop=(cc == CC - 1),
                            )
                    nc.vector.tensor_copy(
                        out=vrow[:, gh * 1024 : (gh + 1) * 1024], in_=vps[:, :]
                    )
                nc.sync.dma_start(out=vfs_d[tb], in_=vrow)
                if tb == 3:
                    nc.gpsimd.collective_compute(
                        "AllGather",
                        mybir.AluOpType.bypass,
                        replica_groups=PAIRS,
                        ins=[vfs_d[0:4, :, :]],
                        outs=[vfallA_d[:, :, :]],
                    )
            nc.gpsimd.collective_compute(
                "AllGather",
                mybir.AluOpType.bypass,
                replica_groups=PAIRS,
                ins=[vfs_d[4:8, :, :]],
                outs=[vfallB_d[:, :, :]],
            )
            # xT loads here: overlap the vf matmuls / AllGather; first needed
            # by the attention score matmuls
            for cc in range(CC):
                nc.sync.dma_start(
                    out=xT[:, cc, :], in_=xT_ext[cc * 128 : (cc + 1) * 128, :]
                )

        # ======== phase 5: attention over owned blocks =====================
        # software-pipelined: block k's scores/softmax run before block k-1's
        # out-matmuls are retired, covering the vf AllGather and the per-block
        # softmax latency with PE work
        with ExitStack() as ph6:
            at = ph6.enter_context(tc.tile_pool(name="at", bufs=1))
            st6 = ph6.enter_context(tc.tile_pool(name="st6", bufs=2))
            vrd = ph6.enter_context(tc.tile_pool(name="vrd", bufs=4))
            small = ph6.enter_context(tc.tile_pool(name="small", bufs=4))
            ps6 = ph6.enter_context(tc.tile_pool(name="ps6", bufs=1, space="PSUM"))

            bfb = at.tile([128, F], F32, tag="bfb")
            bf_ap = bf_ext[:]
            nc.sync.dma_start(
                out=bfb,
                in_=bass.AP(
                    tensor=bf_ap.tensor,
                    offset=bf_ap.offset,
                    ap=[[0, 128]] + list(bf_ap.ap),
                ),
            )
            m2 = at.tile([128, 8, 256], F32, tag="m2")
            for k in range(8):
                nc.sync.dma_start(out=m2[:, k, :], in_=m2_ext[k])

            def softmax_stage(k):
                """scores -> masked SBUF copy -> exp -> transposed bf16 probs"""
                ek = E[k]
                scols = ek * 128
                s_sb = st6.tile([128, T], F32, tag="s_sb", name=f"s_sb{k}")
                for h0 in range(0, scols, 1024):
                    hw = min(1024, scols - h0)
                    sps = ps6.tile([128, 1024], F32, tag="sps", name=f"sps{k}_{h0}")
                    for st in range(0, hw, 512):
                        w = min(512, hw - st)
                        for c2 in range(CC):
                            nc.tensor.matmul(
                                sps[:, st : st + w],
                                uT[:, c2, k * 128 : (k + 1) * 128],
                                xT[:, c2, h0 + st : h0 + st + w],
                                start=(c2 == 0),
                                stop=(c2 == CC - 1),
                            )
                    m0 = scols - 256  # mask window start
                    plain = min(hw, max(0, m0 - h0))
                    if plain > 0:
                        nc.vector.tensor_copy(
                            out=s_sb[:, h0 : h0 + plain], in_=sps[:, :plain]
                        )
                    if plain < hw:
                        nc.vector.tensor_add(
                            s_sb[:, h0 + plain : h0 + hw],
                            sps[:, plain:hw],
                            m2[:, k, h0 + plain - m0 : h0 + hw - m0],
                        )
                negmax = small.tile([128, 1], F32, tag="negmax", name=f"negmax{k}", bufs=6)
                nc.vector.tensor_reduce(
                    out=negmax,
                    in_=s_sb[:, :scols],
                    axis=mybir.AxisListType.X,
                    op=mybir.AluOpType.max,
                    negate=True,
                )
                psb = st6.tile([128, T], BF16, tag="psb", name=f"psb{k}", bufs=5)
                rsum = small.tile([128, 1], F32, tag="rsum", name=f"rsum{k}", bufs=6)
                nc.scalar.activation(
                    out=psb[:, :scols],
                    in_=s_sb[:, :scols],
                    func=mybir.ActivationFunctionType.Exp,
                    bias=negmax,
                    scale=1.0,
                    accum_out=rsum,
                )
                rinv = small.tile([128, 1], F32, tag="rinv", name=f"rinv{k}", bufs=6)
                nc.vector.reciprocal(out=rinv, in_=rsum)
                return psb, rinv

            def out_stage(k, psb, rinv):
                """transpose probs, out = P @ vf (accum over s-chunks), epilogue"""
                ek = E[k]
                ptsb = st6.tile([128, NB, 128], BF16, tag="ptsb", name=f"ptsb{k}")
                for sc in range(ek):
                    pt = ps_t.tile([128, 128], BF16, tag="pt", name=f"pt{k}_{sc}")
                    nc.tensor.transpose(
                        pt[:, :], psb[:, sc * 128 : (sc + 1) * 128], identbf[:, :]
                    )
                    nc.vector.tensor_copy(out=ptsb[:, sc, :], in_=pt[:, :])
                ops = ps6.tile([128, F], F32, tag="ops", name=f"ops{k}")  # 4 banks
                for sc in range(ek):
                    vrow = vrd.tile([128, F], BF16, tag="vread", name=f"vread{k}_{sc}")
                    src = (vfallA_d if (sc % 8) < 4 else vfallB_d)[
                        4 * (sc // 8) + (sc % 4)
                    ]
                    nc.sync.dma_start(out=vrow, in_=src)
                    for g in range(4):
                        nc.tensor.matmul(
                            ops[:, g * 512 : (g + 1) * 512],
                            ptsb[:, sc, :],
                            vrow[:, g * 512 : (g + 1) * 512],
                            start=(sc == 0),
                            stop=(sc == ek - 1),
                        )
                orow = st6.tile([128, F], F32, tag="orow", name=f"orow{k}", bufs=1)
                nc.vector.scalar_tensor_tensor(
                    out=orow,
                    in0=ops,
                    scalar=rinv,
                    in1=bfb,
                    op0=mybir.AluOpType.mult,
                    op1=mybir.AluOpType.add,
                )
                nc.sync.dma_start(out=out_ext[k], in_=orow)

            DEPTH = 4
            staged = {}
            for k in range(DEPTH):
                staged[k] = softmax_stage(k)
            for k in range(8):
                if k + DEPTH < 8:
                    staged[k + DEPTH] = softmax_stage(k + DEPTH)
                out_stage(k, *staged.pop(k))

    nc.finalize()
    return nc


def _get_program():
    if "nc" not in _CACHE:
        _CACHE["nc"] = _build_program()
    return _CACHE["nc"]


def _make_in_maps(x, Wq, Wk, Wv, Wf, bf):
    x = np.ascontiguousarray(x, dtype=np.float32)
    WqT = np.ascontiguousarray(np.asarray(Wq, dtype=np.float32).T)
    WkT = np.ascontiguousarray(np.asarray(Wk, dtype=np.float32).T)
    WvTb = np.ascontiguousarray(np.asarray(Wv, dtype=np.float32).T).astype(
        ml_dtypes.bfloat16
    )
    Wfb = np.asarray(Wf, dtype=np.float32).astype(ml_dtypes.bfloat16)
    bf = np.ascontiguousarray(bf, dtype=np.float32)
    in_maps = []
    for core in range(NCORES):
        b, h = core // 2, core % 2
        own = OWN_H[h]
        xb = x[b]
        xq = np.concatenate([xb[blk * 128 : (blk + 1) * 128] for blk in own], axis=0)
        mask2 = np.zeros((8, 128, 256), dtype=np.float32)
        for k, blk in enumerate(own):
            s0 = (E[k] - 2) * 128  # global key index of mask window start
            s = s0 + np.arange(256)[None, :]
            t = blk * 128 + np.arange(128)[:, None]
            mask2[k] = np.where(s <= t, 0.0, NEG).astype(np.float32)
        xvTb = (
            np.ascontiguousarray(xb[h * 1024 : (h + 1) * 1024].T)
            .astype(ml_dtypes.bfloat16)
        )
        in_maps.append(
            {
                "xTin": np.ascontiguousarray(xb.T),
                "xqTin": np.ascontiguousarray(xq.T),
                "xvTb": xvTb,
                "mask2": mask2,
                "WqT": WqT,
                "WkTs": np.ascontiguousarray(WkT[:, core * 128 : (core + 1) * 128]),
                "WvTb": WvTb,
                "Wfs": np.ascontiguousarray(Wfb[:, core * 256 : (core + 1) * 256]),
                "bf": bf,
            }
        )
    return in_maps


def run_on_hw(inputs, trace=False, trace_cores=None):
    nc = _get_program()
    in_maps = _make_in_maps(**inputs)
    res = run_bass_kernel_spmd(
        nc, in_maps, list(range(NCORES)), trace=trace, trace_cores=trace_cores
    )
    out = np.empty((B, T, F), dtype=np.float32)
    for core in range(NCORES):
        b, h = core // 2, core % 2
        own = OWN_H[h]
        o = res.results[core]["out"]  # [8, 128, F]
        for k, blk in enumerate(own):
            out[b, blk * 128 : (blk + 1) * 128, :] = o[k]
    return out, res


def kernel(x, Wq, Wk, Wv, Wf, bf):
    out, _ = run_on_hw(dict(x=x, Wq=Wq, Wk=Wk, Wv=Wv, Wf=Wf, bf=bf))
    return out
